# revision 21
# baseline (speedup 1.0000x reference)
"""Trainium2 Bass kernel v3 for causal multi-head self-attention.

Problem (hardcoded):
    x:      [2, 2048, 1024] f32
    W_qkv:  [1024, 3072] f32   (cols: [q | k | v], each 1024 = 16 heads x 64)
    b_qkv:  [3072] f32
    W_proj: [1024, 1024] f32
    b_proj: [1024] f32
    out:    [2, 2048, 1024] f32

Sharding over 8 NeuronCores: data parallel on batch (2) x tensor parallel on
heads (4 quads of 4 heads). Core c handles batch c//4, heads [4*(c%4), 4*(c%4)+4).
Host gather sums the 4 partial projections per batch and adds b_proj.

v3 changes vs v2:
  - AV matmul orientation flipped: out[q,hd] accumulators with exp weights as
    the stationary operand and the 65-col V(+ones) as the moving operand.
    Halves AV's PE column charge (the cost model charges moving columns only).
  - softmax normalize becomes a per-partition-scalar multiply (reciprocal of
    the ones-column sums + tensor_scalar_mul on the PSUM->SBUF copy); the
    DRAM-bounce broadcast, gpsimd multiplies, and scratch tensor are gone.
  - a cheap PE transpose (identity matmul, 128 cols/tile) restores the
    [hd, q] layout the out-projection consumes; pair-packed attn2 unchanged.
  - last block's tail is pipelined per 128-query subtile: each AV chain stops
    early, normalizes, transposes, and feeds its projection m-tile while the
    remaining chains still accumulate.
"""

import os
import sys

for _p in ("/opt/trn_rl_repo", "/root/.axon_site/_ro/trn_rl_repo"):
    if os.path.isdir(_p) and _p not in sys.path:
        sys.path.append(_p)

import numpy as np

import concourse.bass as bass
import concourse.mybir as mybir
import concourse.tile as tile
from concourse.alu_op_type import AluOpType

F32 = mybir.dt.float32
BF16 = mybir.dt.bfloat16
AFT = mybir.ActivationFunctionType

B, S, D, H, HD = 2, 2048, 1024, 16, 64
NCORES = 8
NH = 4  # heads per core
SCALE = 1.0 / 8.0  # 1/sqrt(64)


class SplitWaitTileContext(tile.TileContext):
    """This container's walrus rejects >1 sync wait per instruction
    ("Too many sync wait commands"). Split extra waits onto preceding
    same-engine NoOps before the final block lowering."""

    def _lower_ordered_insts(self, ordered):
        for bb_name, insts in list(ordered.items()):
            new = []
            for inst in insts:
                si = inst.sync_info
                if si is not None and si.on_wait and len(si.on_wait) > 1:
                    waits = list(si.on_wait)
                    for w in waits[:-1]:
                        nop = mybir.InstNoOp(
                            name=f"nopw-{self.nc.get_next_instruction_name()}"
                        )
                        nop.engine = inst.engine
                        nop.sync_info = mybir.SyncInfo(on_wait=[w], on_update=[])
                        new.append(nop)
                    inst.sync_info = mybir.SyncInfo(
                        on_wait=[waits[-1]], on_update=list(si.on_update or [])
                    )
                new.append(inst)
            ordered[bb_name] = new
        return super()._lower_ordered_insts(ordered)

    def _drain_and_barrier(self, tick_clock, wait_clock):
        from concourse.vector_clock import ScopedClock

        drain_inst = self.nc.sync.drain()
        wait_clock.add_sem_waits(
            drain_inst.ins, ScopedClock({None: tick_clock.global_clock})
        )
        si = drain_inst.ins.sync_info
        if si is not None and si.on_wait and len(si.on_wait) > 1:
            waits = list(si.on_wait)
            drain_inst.ins.sync_info = mybir.SyncInfo(
                on_wait=[waits[0]], on_update=list(si.on_update or [])
            )
            for w in waits[1:]:
                nop = self.nc.sync.nop(nofuse=True)
                nop.ins.sync_info = mybir.SyncInfo(on_wait=[w], on_update=[])

        self.nc.all_engine_barrier()
        assert self.sems is not None
        popped = self.nc._tile_sem_poison_stack.pop()
        assert popped is self._sem_poison
        self.nc.clear_and_free_semaphores(list(self.sems.allocated().values()))
        self.nc.all_engine_barrier()


def build_nc(S=S, D=D, NH=NH, dbg=False, reps=1):
    """Build the single-core SPMD program."""
    KD = D // 128             # 8 k-chunks of the D contraction
    NM = NH * 2 * 64 // 128   # 4 qk M-tiles (q chunks then k chunks)
    NMQ = NM // 2
    SQB = S // 512            # 4 sq blocks of 512
    NSK = S // 128            # 16 sk tiles of 128
    NPAIR = NH // 2           # 2 head pairs per block
    NPJ = SQB * NPAIR         # 8 pair-slots (pair index pi = 2*j + pp)
    MISC_W = NM + NH * 64 + 128 + 128  # bqk | bvbc | mask | identity

    nc = bass.Bass("TRN2", target_bir_lowering=False, debug=False)

    xT_d = nc.dram_tensor("xT", [D, S], BF16, kind="ExternalInput").ap()
    wqk_d = nc.dram_tensor("wqk", [D, NM * 128], BF16, kind="ExternalInput").ap()
    wv_d = nc.dram_tensor("wv", [D, NH * 64], BF16, kind="ExternalInput").ap()
    misc_d = nc.dram_tensor("misc", [128, MISC_W], F32, kind="ExternalInput").ap()
    # wproj pre-packed on host: [p, pp, n] = W_proj[core_base + pp*128 + p, n]
    wproj_d = nc.dram_tensor("wproj", [128, NPAIR, D], BF16, kind="ExternalInput").ap()
    y_d = nc.dram_tensor("y", [S, D], BF16, kind="ExternalOutput").ap()

    with SplitWaitTileContext(nc) as tc:
        with (
            nc.allow_low_precision(reason="bf16 feeds PE; fp32 accum in PSUM"),
            tc.tile_pool(name="stream", bufs=2) as p_stream,
            tc.tile_pool(name="attnp", bufs=1) as p_attn,
            tc.tile_pool(name="wpool", bufs=1) as p_w,
            tc.tile_pool(name="qkt", bufs=1) as p_qkt,
            tc.tile_pool(name="vaug", bufs=1) as p_vaug,
            tc.tile_pool(name="expp", bufs=8) as p_exp,
            tc.tile_pool(name="asb", bufs=2) as p_asb,
            tc.tile_pool(name="rcpp", bufs=2) as p_rcp,
            tc.tile_pool(name="ypool", bufs=8) as p_y,
            tc.tile_pool(name="pmisc", bufs=2, space="PSUM") as p_misc,
            tc.tile_pool(name="ps", bufs=2, space="PSUM") as p_s,
            tc.tile_pool(name="pavq", bufs=1, space="PSUM") as p_avq,
            tc.tile_pool(name="ppst", bufs=1, space="PSUM") as p_pst,
        ):
          for _rep in range(reps):
            # PE warmup: junk matmuls keep the systolic array ramped while the
            # input DMAs land
            ones_sb = p_w.tile([128, 260], BF16, tag="ones")
            nc.vector.memset(ones_sb[:, :], 1.0)
            # preload the exp table set in the startup window
            expwarm = p_w.tile([1, 1], F32, tag="expwarm")
            nc.scalar.activation(
                expwarm[:, :], ones_sb[0:1, 0:1], AFT.Exp, scale=SCALE
            )
            warm_ps = p_avq.tile([128, 4, 65], F32, tag="avq", name="warm_ps")

            def junk(n):
                # junk output spans the full AV-chain byte range so the PSUM
                # pending-zero flags its start=True raises (2KB-aligned) are
                # all cleared by its own write
                for _ in range(n):
                    nc.tensor.matmul(
                        warm_ps[0:64, :, :],
                        lhsT=ones_sb[:, 0:64],
                        rhs=ones_sb[:, :],
                        start=True,
                        stop=True,
                    )

            junk(40)

            # input DMAs: transfers AND descriptor-gen (627ns HWDGE) serialize
            # device-wide, so the first-needed big transfers go first; tiny
            # bias/mask loads ride behind them (needed only ~10us in)
            xT_src = xT_d.rearrange("(c p) s -> p c s", p=128)
            xs0 = p_stream.tile([128, KD, 512], BF16, tag="xs")
            wqk_sb = p_w.tile([128, KD, NM * 128], BF16, tag="wqk")
            wqk_src = wqk_d.rearrange("(c p) n -> p c n", p=128)
            # 4-chunk granularity balances serialized HWDGE descriptor-gen
            # against time-to-first-chunk
            KH = KD // 2
            nc.sync.dma_start(out=xs0[:, 0:KH, :], in_=xT_src[:, 0:KH, 0:512])
            nc.sync.dma_start(out=wqk_sb[:, 0:KH, :], in_=wqk_src[:, 0:KH, :])
            nc.sync.dma_start(out=xs0[:, KH:KD, :], in_=xT_src[:, KH:KD, 0:512])
            nc.sync.dma_start(out=wqk_sb[:, KH:KD, :], in_=wqk_src[:, KH:KD, :])

            misc_sb = p_w.tile([128, MISC_W], F32, tag="misc")
            nc.sync.dma_start(out=misc_sb[:, :], in_=misc_d[:, :])
            bqk_sb = misc_sb[:, 0:NM]
            bvbc_sb = misc_sb[:, NM:NM + NH * 64]
            mask_sb = p_w.tile([128, 128], BF16, tag="mask")
            nc.vector.tensor_copy(
                mask_sb[:, :], misc_sb[:, NM + NH * 64:NM + NH * 64 + 128]
            )
            ident_sb = p_w.tile([128, 128], BF16, tag="ident")
            nc.vector.tensor_copy(
                ident_sb[:, :], misc_sb[:, NM + NH * 64 + 128:MISC_W]
            )

            wv_sb = p_w.tile([128, KD, NH * 64], BF16, tag="wv")
            wv_src = wv_d.rearrange("(c p) n -> p c n", p=128)
            nc.sync.dma_start(out=wv_sb[:, :, :], in_=wv_src[:, :, :])

            xs1 = p_stream.tile([128, KD, 512], BF16, tag="xs", name="xs1")
            nc.sync.dma_start(out=xs1[:, :, :], in_=xT_src[:, :, 512:1024])

            # wproj is only needed by the projection fillers in the final
            # block (~90us in); load it behind everything the front needs
            wproj_sb = p_w.tile([128, NPAIR, D], BF16, tag="wproj")
            nc.sync.dma_start(out=wproj_sb[:, :, :], in_=wproj_d[:, :, :])

            qkT_sb = p_qkt.tile([128, NM, S], BF16, tag="qkt")
            v_aug = p_vaug.tile([128, NSK, NH, 65], BF16, tag="vaug")
            nc.vector.memset(v_aug[:, :, :, 64:65], 1.0)
            # attn2: pair-packed normalized attnT. partitions 0:64 head 2pp,
            # 64:128 head 2pp+1; slot pi = 2*j + pp
            attn2 = p_attn.tile([128, NPJ, 512], BF16, tag="attn")

            def load_xs(j):
                xs = p_stream.tile([128, KD, 512], BF16, tag="xs")
                nc.sync.dma_start(
                    out=xs[:, :, :], in_=xT_src[:, :, j * 512:(j + 1) * 512]
                )
                return xs

            def qk_move(j, mp, ps_qk):
                dst = qkT_sb[:, mp, j * 512:(j + 1) * 512]
                nc.vector.tensor_scalar_add(dst, ps_qk[:, :], bqk_sb[:, mp:mp + 1])

            def qk_steps(j, xs, mp):
                """Micro-steps (one matmul each) for one qk projection tile."""
                cell = {}

                def mm(k):
                    if k == 0:
                        cell["ps"] = p_misc.tile([128, 512], F32, tag="m", name="ps_qk")
                    nc.tensor.matmul(
                        cell["ps"][:, :],
                        lhsT=wqk_sb[:, k, mp * 128:(mp + 1) * 128],
                        rhs=xs[:, k, :],
                        start=(k == 0),
                        stop=(k == KD - 1),
                    )

                return [(True, lambda k=k: mm(k)) for k in range(KD)] + [
                    (False, lambda: qk_move(j, mp, cell["ps"]))
                ]

            def v_steps(j, xs, m):
                cell = {}

                def mm(k):
                    if k == 0:
                        cell["ps"] = p_misc.tile([128, NH * 64], F32, tag="m", name="ps_v")
                    nc.tensor.matmul(
                        cell["ps"][:, :],
                        lhsT=xs[:, k, (m % 4) * 128:(m % 4) * 128 + 128],
                        rhs=wv_sb[:, k, :],
                        start=(k == 0),
                        stop=(k == KD - 1),
                    )

                def mv():
                    nc.vector.tensor_add(
                        v_aug[:, m, :, 0:64],
                        cell["ps"][:, :].rearrange("p (h c) -> p h c", c=64),
                        bvbc_sb.rearrange("p (h c) -> p h c", c=64),
                    )

                return [(True, lambda k=k: mm(k)) for k in range(KD)] + [
                    (False, mv)
                ]

            def qkv_steps(j, xs):
                steps = []
                for mp in range(NM):
                    steps += qk_steps(j, xs, mp)
                for m in range(4 * j, 4 * j + 4):
                    steps += v_steps(j, xs, m)
                return steps

            def attention_block(j, fillers):
                # in the final block, hold back a few PE filler steps for the
                # ACT-paced last-head stretch
                reserve = [4 if j == SQB - 1 else 0]
                # per-group filler quota: qkv steps are 213ns, proj steps
                # (atomic 2-matmul chains) are 426ns; ACT outruns this loop's
                # PE work by ~350ns/group
                pump_n = 1 if j == SQB - 1 else 2

                def pump(n=1):
                    got = 0
                    while fillers and got < n:
                        if reserve[0] and sum(
                            1 for p, _ in fillers if p
                        ) <= reserve[0]:
                            return
                        is_pe, fn = fillers.pop(0)
                        fn()
                        if is_pe:
                            got += 1

                for h in range(NH):
                    qT = qkT_sb[64 * (h % 2):64 * (h % 2) + 64, h // 2, :]
                    kT = qkT_sb[64 * (h % 2):64 * (h % 2) + 64, NMQ + h // 2, :]
                    ps_avq = p_avq.tile([128, 4, 65], F32, tag="avq")
                    # the 4 AV chains share one PSUM bank; a matmul start=True
                    # would mark the whole 2KB bank pending-zero and wipe its
                    # neighbours' partials, so zero the bank once on DVE and
                    # accumulate with start=False throughout
                    nc.vector.memset(ps_avq[:, :, :], 0.0)
                    pp = h // 2
                    pi = 2 * j + pp
                    if h % 2 == 0:
                        asb_cur = p_asb.tile([128, 4, 128], BF16, tag="asb")
                    asb = asb_cur
                    npair = 2 * (j + 1)
                    tail = j == SQB - 1 and h == NH - 1

                    def noff(i):
                        # causal column truncation (bf16: no N>=256 minimum)
                        mb = i - 4 * j
                        return 0 if mb <= 0 else 128 * mb

                    def emit_scores(g):
                        # exact causal regions; the merged diag exp also reads
                        # stale PSUM outside them, which downstream AV never
                        # consumes (harmless garbage, skipped per subtile)
                        ps = p_s.tile([128, 2, 512], F32, tag="s")
                        for b in range(2):
                            i = 2 * g + b
                            no = noff(i)
                            nc.tensor.matmul(
                                ps[:, b, no:512],
                                lhsT=kT[:, i * 128:(i + 1) * 128],
                                rhs=qT[:, j * 512 + no:(j + 1) * 512],
                                start=True,
                                stop=True,
                            )
                        return ps

                    rcp = p_rcp.tile([128, 4, 1], F32, tag="rcp")

                    def norm_subtile(t):
                        """reciprocal of the ones-column sum + normalized
                        PSUM->SBUF copy for q-subtile t of this head."""
                        nc.vector.reciprocal(
                            rcp[:, t:t + 1, :], ps_avq[:, t:t + 1, 64:65]
                        )
                        nc.vector.tensor_scalar_mul(
                            asb[:, t, 64 * (h % 2):64 * (h % 2) + 64],
                            ps_avq[:, t, 0:64],
                            rcp[:, t, :],
                        )

                    def tail_subtile(t, psT):
                        """Last head: chain t stopped one b-step ago and its
                        normalize already ran on DVE. Transpose and close
                        projection m-tile 12+t (both 512-col halves)."""
                        nc.tensor.transpose(
                            psT[:, t, :], asb[:, t, :], ident_sb[:, :]
                        )
                        nc.vector.tensor_copy(
                            attn2[:, pi, 128 * t:128 * (t + 1)], psT[:, t, :]
                        )
                        y_sb = p_y.tile([128, 2, 512], BF16, tag="y",
                                        name="y_sb")
                        chs = []
                        for n in range(2):
                            ch = p_misc.tile([128, 512], F32, tag="m",
                                             name="tp_ps")
                            chs.append(ch)
                            nc.tensor.matmul(
                                ch[:, :],
                                lhsT=attn2[:, 2 * j, t * 128:(t + 1) * 128],
                                rhs=wproj_sb[:, 0, n * 512:(n + 1) * 512],
                                start=True,
                                stop=False,
                            )
                        for n in range(2):
                            nc.tensor.matmul(
                                chs[n][:, :],
                                lhsT=attn2[:, 2 * j + 1,
                                           t * 128:(t + 1) * 128],
                                rhs=wproj_sb[:, 1, n * 512:(n + 1) * 512],
                                start=False,
                                stop=True,
                            )
                        m = 4 * j + t
                        nc.scalar.copy(y_sb[:, 0, :], chs[0][:, :])
                        if t == 3:
                            # final m-tile: half DMAs overlap the second
                            # half's copy with the first half's transfer
                            nc.sync.dma_start(
                                out=y_d[m * 128:(m + 1) * 128, 0:512],
                                in_=y_sb[:, 0, :],
                            )
                            nc.vector.tensor_copy(y_sb[:, 1, :], chs[1][:, :])
                            nc.sync.dma_start(
                                out=y_d[m * 128:(m + 1) * 128, 512:1024],
                                in_=y_sb[:, 1, :],
                            )
                        else:
                            nc.vector.tensor_copy(y_sb[:, 1, :], chs[1][:, :])
                            nc.sync.dma_start(
                                out=y_d[m * 128:(m + 1) * 128, :],
                                in_=y_sb[:, :, :],
                            )

                    sc_next = emit_scores(0)
                    if tail:
                        psT = p_pst.tile([128, 4, 128], BF16, tag="pst")
                    for g in range(npair):
                        ps_sc = sc_next
                        # 1-deep software pipeline: next group's scores are
                        # emitted before this group's AV so PE runs them
                        # while ACT computes this group's exp
                        if g + 1 < npair:
                            sc_next = emit_scores(g + 1)
                        # ACT runs ~350ns/group longer than this loop's PE
                        # work; pump filler steps so PE never idles on exp
                        pump(pump_n)
                        exp_t = p_exp.tile([128, 2, 512], BF16, tag="exp")
                        if g == 2 * j:
                            # diag pair mb=0,1: single exp over both tiles
                            nc.scalar.activation(
                                exp_t[:, :, :], ps_sc[:, :, :], AFT.Exp, scale=SCALE
                            )
                            nc.vector.tensor_mul(
                                exp_t[:, 0, 0:128], exp_t[:, 0, 0:128], mask_sb[:, :]
                            )
                            nc.vector.tensor_mul(
                                exp_t[:, 1, 128:256], exp_t[:, 1, 128:256],
                                mask_sb[:, :],
                            )
                        elif g == 2 * j + 1:
                            # mb=2,3: exp the computed 256:512 of both tiles
                            nc.scalar.activation(
                                exp_t[:, :, 256:512],
                                ps_sc[:, :, 256:512],
                                AFT.Exp,
                                scale=SCALE,
                            )
                            nc.vector.tensor_mul(
                                exp_t[:, 0, 256:384], exp_t[:, 0, 256:384],
                                mask_sb[:, :],
                            )
                            nc.vector.tensor_mul(
                                exp_t[:, 1, 384:512], exp_t[:, 1, 384:512],
                                mask_sb[:, :],
                            )
                        else:
                            nc.scalar.activation(
                                exp_t[:, :, :], ps_sc[:, :, :], AFT.Exp, scale=SCALE
                            )
                        for b in range(2):
                            i = 2 * g + b
                            mb = i - 4 * j
                            for t in range(max(0, mb), 4):
                                nc.tensor.matmul(
                                    ps_avq[:, t, :],
                                    lhsT=exp_t[:, b, 128 * t:128 * (t + 1)],
                                    rhs=v_aug[:, i, h, :],
                                    start=False,
                                    stop=(i == 4 * j + t),
                                    skip_group_check=True,
                                )
                            if tail and mb >= 0:
                                # chain mb just stopped: normalize on DVE now
                                norm_subtile(mb)
                            if tail and mb >= 1:
                                # chain mb-1 normalized one b-step ago ->
                                # transpose + close its projection m-tile
                                tail_subtile(mb - 1, psT)
                    if tail:
                        # drain remaining fillers (their y DMAs must precede
                        # the final m-tile's), then close the last subtile
                        reserve[0] = 0
                        while fillers:
                            fillers.pop(0)[1]()
                        tail_subtile(3, psT)
                    elif h % 2 == 0:
                        for t in range(4):
                            norm_subtile(t)
                    else:
                        psT = p_pst.tile([128, 4, 128], BF16, tag="pst")
                        for t in range(4):
                            norm_subtile(t)
                        # fillers between the DVE normalizes and the PE
                        # transposes hide the normalize latency
                        pump(2)
                        for t in range(4):
                            nc.tensor.transpose(
                                psT[:, t, :], asb[:, t, :], ident_sb[:, :]
                            )
                        nc.vector.tensor_copy(attn2[:, pi, :], psT[:, :, :])
                    # drain PE filler work into the ACT-paced stretch,
                    # counting only PE (matmul) steps toward the quota
                    if h >= 1 and not tail:
                        npe = sum(1 for is_pe, _ in fillers if is_pe)
                        take = max(1, (npe - reserve[0]) // (6 * (NH - h)))
                        while fillers and take > 0:
                            if reserve[0] and sum(
                                1 for p, _ in fillers if p
                            ) <= reserve[0]:
                                break
                            is_pe, fn = fillers.pop(0)
                            fn()
                            if is_pe:
                                take -= 1
                while fillers:
                    fillers.pop(0)[1]()

            def proj_steps_m(j, m):
                """Micro-steps for one 128-row tile of the out-projection.
                Each 512-col chain (both pair accumulations) is one atomic
                step so a pump boundary never leaves a PSUM chain open while
                other code allocates from the same pool."""
                o = (m % 4) * 128
                cell = {}

                def mmv(n):
                    if n == 0:
                        cell["y"] = p_y.tile([128, 2, 512], BF16, tag="y",
                                             name="y_sb")
                    ps = p_misc.tile([128, 512], F32, tag="m", name="ps_y")
                    for pp in range(NPAIR):
                        nc.tensor.matmul(
                            ps[:, :],
                            lhsT=attn2[:, 2 * j + pp, o:o + 128],
                            rhs=wproj_sb[:, pp, n * 512:(n + 1) * 512],
                            start=(pp == 0),
                            stop=(pp == NPAIR - 1),
                        )
                    nc.vector.tensor_copy(cell["y"][:, n, :], ps[:, :])

                def out():
                    nc.sync.dma_start(
                        out=y_d[m * 128:(m + 1) * 128, :],
                        in_=cell["y"][:, :, :],
                    )

                steps = []
                for n in range(2):
                    steps.append((True, lambda n=n: mmv(n)))
                steps.append((False, out))
                return steps

            def proj_steps(j):
                steps = []
                for m in range(j * 4, j * 4 + 4):
                    steps += proj_steps_m(j, m)
                return steps

            # j=0 prologue. The startup is DMA-serial-bound: run all four qk
            # tiles chunk-half-major (4 concurrent PSUM chains, borrowing the
            # idle score pool) so PE tracks the half-chunk DMA cadence; then
            # v chunk-major the same way.
            junk(75)
            ps_qk0 = p_misc.tile([128, 512], F32, tag="m")
            ps_qk1 = p_misc.tile([128, 512], F32, tag="m")
            ps_qk23 = p_s.tile([128, 2, 512], F32, tag="s")
            chains = (ps_qk0[:, :], ps_qk1[:, :], ps_qk23[:, 0, :],
                      ps_qk23[:, 1, :])
            for kh in range(2):
                for k in range(kh * KH, (kh + 1) * KH):
                    for mp in range(NM):
                        nc.tensor.matmul(
                            chains[mp],
                            lhsT=wqk_sb[:, k, mp * 128:(mp + 1) * 128],
                            rhs=xs0[:, k, :],
                            start=(k == 0),
                            stop=(k == KD - 1),
                        )
            for mp in range(NM):
                qk_move(0, mp, chains[mp])
            ps_v01 = p_s.tile([128, 2, 512], F32, tag="s")
            vchains = (ps_v01[:, 0, 0:256], ps_v01[:, 1, 0:256], None, None)
            vcells = [None, None, None, None]
            for k in range(KD):
                for m in range(4):
                    if m < 2:
                        ps = vchains[m]
                    else:
                        if k == 0 and vcells[m] is None:
                            vcells[m] = p_misc.tile(
                                [128, NH * 64], F32, tag="m", name="ps_v"
                            )
                        ps = vcells[m][:, :]
                    nc.tensor.matmul(
                        ps,
                        lhsT=xs0[:, k, m * 128:m * 128 + 128],
                        rhs=wv_sb[:, k, :],
                        start=(k == 0),
                        stop=(k == KD - 1),
                    )
            for m in range(4):
                src = vchains[m] if m < 2 else vcells[m][:, :]
                nc.vector.tensor_add(
                    v_aug[:, m, :, 0:64],
                    src.rearrange("p (h c) -> p h c", c=64),
                    bvbc_sb.rearrange("p (h c) -> p h c", c=64),
                )
            xs_next = xs1 if SQB > 1 else None
            for j in range(SQB):
                fillers = []
                if j + 1 < SQB:
                    fillers += qkv_steps(j + 1, xs_next)
                    xs_after = load_xs(j + 2) if j + 2 < SQB else None
                else:
                    xs_after = None
                if j == SQB - 1:
                    for jp in range(SQB - 1):
                        fillers += proj_steps(jp)
                attention_block(j, fillers)
                xs_next = xs_after

    return nc


def make_mask4():
    p = np.arange(128)[:, None]
    f = np.arange(128)[None, :]
    return (f >= p).astype(np.float32).copy()  # [128, 128] lower-tri in T layout


def to_bf16(x):
    import ml_dtypes

    return np.asarray(x, dtype=np.float32).astype(ml_dtypes.bfloat16)


def make_in_maps(x, W_qkv, b_qkv, W_proj):
    """Per-core input dicts for the full-size problem (bf16 staged)."""
    mask4 = make_mask4()
    ident = np.eye(128, dtype=np.float32)
    in_maps = []
    for c in range(NCORES):
        b, q = c // 4, c % 4
        cq = slice(256 * q, 256 * q + 256)
        wqk = np.concatenate([W_qkv[:, cq], W_qkv[:, 1024:2048][:, cq]], axis=1)
        wv = W_qkv[:, 2048:3072][:, cq]
        bqk = np.concatenate([b_qkv[cq], b_qkv[1024:2048][cq]]).reshape(4, 128)
        bvbc = np.broadcast_to(b_qkv[2048:3072][cq], (128, 256))
        # packed misc input: [128, 4 bqk-cols | 256 bvbc | 128 mask | 128 id]
        misc = np.concatenate([bqk.T, bvbc, mask4, ident], axis=1).astype(
            np.float32
        )
        # pair-packed wproj: [p, pp, n] = W_proj[256*q + pp*128 + p, n]
        wproj = np.ascontiguousarray(
            W_proj[cq, :].reshape(2, 128, 1024).transpose(1, 0, 2)
        )
        in_maps.append(
            {
                "xT": np.ascontiguousarray(to_bf16(x[b].T)),
                "wqk": np.ascontiguousarray(to_bf16(wqk)),
                "wv": np.ascontiguousarray(to_bf16(wv)),
                "misc": np.ascontiguousarray(misc),
                "wproj": to_bf16(wproj),
            }
        )
    return in_maps


_NC_CACHE = {}


def _get_nc():
    if "nc" not in _NC_CACHE:
        _NC_CACHE["nc"] = build_nc()
    return _NC_CACHE["nc"]


def run_on_hw(x, W_qkv, b_qkv, W_proj, b_proj, trace=False, **trace_kw):
    from concourse.bass_utils import run_bass_kernel_spmd

    in_maps = make_in_maps(x, W_qkv, b_qkv, W_proj)
    res = run_bass_kernel_spmd(
        _get_nc(), in_maps, core_ids=list(range(NCORES)), trace=trace, **trace_kw
    )
    out = np.empty((B, S, D), dtype=np.float32)
    for b in range(B):
        acc = res.results[4 * b]["y"].astype(np.float32)
        for q in range(1, 4):
            acc = acc + res.results[4 * b + q]["y"].astype(np.float32)
        out[b] = acc + b_proj[None, :]
    return out, res


def kernel(x, W_qkv, b_qkv, W_proj, b_proj):
    x = np.asarray(x, dtype=np.float32)
    W_qkv = np.asarray(W_qkv, dtype=np.float32)
    b_qkv = np.asarray(b_qkv, dtype=np.float32)
    W_proj = np.asarray(W_proj, dtype=np.float32)
    b_proj = np.asarray(b_proj, dtype=np.float32)
    out, _ = run_on_hw(x, W_qkv, b_qkv, W_proj, b_proj, trace=False)
    return out


# revision 23
# speedup vs baseline: 1.0010x; 1.0010x over previous
"""Trainium2 Bass kernel v3 for causal multi-head self-attention.

Problem (hardcoded):
    x:      [2, 2048, 1024] f32
    W_qkv:  [1024, 3072] f32   (cols: [q | k | v], each 1024 = 16 heads x 64)
    b_qkv:  [3072] f32
    W_proj: [1024, 1024] f32
    b_proj: [1024] f32
    out:    [2, 2048, 1024] f32

Sharding over 8 NeuronCores: data parallel on batch (2) x tensor parallel on
heads (4 quads of 4 heads). Core c handles batch c//4, heads [4*(c%4), 4*(c%4)+4).
Host gather sums the 4 partial projections per batch and adds b_proj.

v3 changes vs v2:
  - AV matmul orientation flipped: out[q,hd] accumulators with exp weights as
    the stationary operand and the 65-col V(+ones) as the moving operand.
    Halves AV's PE column charge (the cost model charges moving columns only).
  - softmax normalize becomes a per-partition-scalar multiply (reciprocal of
    the ones-column sums + tensor_scalar_mul on the PSUM->SBUF copy); the
    DRAM-bounce broadcast, gpsimd multiplies, and scratch tensor are gone.
  - a cheap PE transpose (identity matmul, 128 cols/tile) restores the
    [hd, q] layout the out-projection consumes; pair-packed attn2 unchanged.
  - last block's tail is pipelined per 128-query subtile: each AV chain stops
    early, normalizes, transposes, and feeds its projection m-tile while the
    remaining chains still accumulate.
"""

import os
import sys

for _p in ("/opt/trn_rl_repo", "/root/.axon_site/_ro/trn_rl_repo"):
    if os.path.isdir(_p) and _p not in sys.path:
        sys.path.append(_p)

import numpy as np

import concourse.bass as bass
import concourse.mybir as mybir
import concourse.tile as tile
from concourse.alu_op_type import AluOpType

F32 = mybir.dt.float32
BF16 = mybir.dt.bfloat16
AFT = mybir.ActivationFunctionType

B, S, D, H, HD = 2, 2048, 1024, 16, 64
NCORES = 8
NH = 4  # heads per core
SCALE = 1.0 / 8.0  # 1/sqrt(64)


class SplitWaitTileContext(tile.TileContext):
    """This container's walrus rejects >1 sync wait per instruction
    ("Too many sync wait commands"). Split extra waits onto preceding
    same-engine NoOps before the final block lowering."""

    def _lower_ordered_insts(self, ordered):
        for bb_name, insts in list(ordered.items()):
            new = []
            for inst in insts:
                si = inst.sync_info
                if si is not None and si.on_wait and len(si.on_wait) > 1:
                    waits = list(si.on_wait)
                    for w in waits[:-1]:
                        nop = mybir.InstNoOp(
                            name=f"nopw-{self.nc.get_next_instruction_name()}"
                        )
                        nop.engine = inst.engine
                        nop.sync_info = mybir.SyncInfo(on_wait=[w], on_update=[])
                        new.append(nop)
                    inst.sync_info = mybir.SyncInfo(
                        on_wait=[waits[-1]], on_update=list(si.on_update or [])
                    )
                new.append(inst)
            ordered[bb_name] = new
        return super()._lower_ordered_insts(ordered)

    def _drain_and_barrier(self, tick_clock, wait_clock):
        from concourse.vector_clock import ScopedClock

        drain_inst = self.nc.sync.drain()
        wait_clock.add_sem_waits(
            drain_inst.ins, ScopedClock({None: tick_clock.global_clock})
        )
        si = drain_inst.ins.sync_info
        if si is not None and si.on_wait and len(si.on_wait) > 1:
            waits = list(si.on_wait)
            drain_inst.ins.sync_info = mybir.SyncInfo(
                on_wait=[waits[0]], on_update=list(si.on_update or [])
            )
            for w in waits[1:]:
                nop = self.nc.sync.nop(nofuse=True)
                nop.ins.sync_info = mybir.SyncInfo(on_wait=[w], on_update=[])

        self.nc.all_engine_barrier()
        assert self.sems is not None
        popped = self.nc._tile_sem_poison_stack.pop()
        assert popped is self._sem_poison
        self.nc.clear_and_free_semaphores(list(self.sems.allocated().values()))
        self.nc.all_engine_barrier()


def build_nc(S=S, D=D, NH=NH, dbg=False, reps=1):
    """Build the single-core SPMD program."""
    KD = D // 128             # 8 k-chunks of the D contraction
    NM = NH * 2 * 64 // 128   # 4 qk M-tiles (q chunks then k chunks)
    NMQ = NM // 2
    SQB = S // 512            # 4 sq blocks of 512
    NSK = S // 128            # 16 sk tiles of 128
    NPAIR = NH // 2           # 2 head pairs per block
    NPJ = SQB * NPAIR         # 8 pair-slots (pair index pi = 2*j + pp)
    MISC_W = NM + NH * 64 + 128 + 128  # bqk | bvbc | mask | identity

    nc = bass.Bass("TRN2", target_bir_lowering=False, debug=False)

    xT_d = nc.dram_tensor("xT", [D, S], BF16, kind="ExternalInput").ap()
    wqk_d = nc.dram_tensor("wqk", [D, NM * 128], BF16, kind="ExternalInput").ap()
    wv_d = nc.dram_tensor("wv", [D, NH * 64], BF16, kind="ExternalInput").ap()
    misc_d = nc.dram_tensor("misc", [128, MISC_W], F32, kind="ExternalInput").ap()
    # wproj pre-packed on host: [p, pp, n] = W_proj[core_base + pp*128 + p, n]
    wproj_d = nc.dram_tensor("wproj", [128, NPAIR, D], BF16, kind="ExternalInput").ap()
    y_d = nc.dram_tensor("y", [S, D], BF16, kind="ExternalOutput").ap()

    with SplitWaitTileContext(nc) as tc:
        with (
            nc.allow_low_precision(reason="bf16 feeds PE; fp32 accum in PSUM"),
            tc.tile_pool(name="stream", bufs=2) as p_stream,
            tc.tile_pool(name="attnp", bufs=1) as p_attn,
            tc.tile_pool(name="wpool", bufs=1) as p_w,
            tc.tile_pool(name="qkt", bufs=1) as p_qkt,
            tc.tile_pool(name="vaug", bufs=1) as p_vaug,
            tc.tile_pool(name="expp", bufs=8) as p_exp,
            tc.tile_pool(name="asb", bufs=2) as p_asb,
            tc.tile_pool(name="rcpp", bufs=2) as p_rcp,
            tc.tile_pool(name="ypool", bufs=8) as p_y,
            tc.tile_pool(name="pmisc", bufs=2, space="PSUM") as p_misc,
            tc.tile_pool(name="ps", bufs=2, space="PSUM") as p_s,
            tc.tile_pool(name="pavq", bufs=1, space="PSUM") as p_avq,
            tc.tile_pool(name="ppst", bufs=1, space="PSUM") as p_pst,
        ):
          for _rep in range(reps):
            # PE warmup: junk matmuls keep the systolic array ramped while the
            # input DMAs land
            ones_sb = p_w.tile([128, 260], BF16, tag="ones")
            nc.vector.memset(ones_sb[:, :], 1.0)
            zero_sb = p_w.tile([128, 128], BF16, tag="zero")
            nc.vector.memset(zero_sb[:, :], 0.0)
            # preload the exp table set in the startup window
            expwarm = p_w.tile([1, 1], F32, tag="expwarm")
            nc.scalar.activation(
                expwarm[:, :], ones_sb[0:1, 0:1], AFT.Exp, scale=SCALE
            )
            warm_ps = p_avq.tile([128, 4, 65], F32, tag="avq", name="warm_ps")

            def junk(n):
                # junk output spans the full AV-chain byte range so the PSUM
                # pending-zero flags its start=True raises (2KB-aligned) are
                # all cleared by its own write
                for _ in range(n):
                    nc.tensor.matmul(
                        warm_ps[0:64, :, :],
                        lhsT=ones_sb[:, 0:64],
                        rhs=ones_sb[:, :],
                        start=True,
                        stop=True,
                    )

            junk(40)

            # input DMAs: transfers AND descriptor-gen (627ns HWDGE) serialize
            # device-wide, so the first-needed big transfers go first; tiny
            # bias/mask loads ride behind them (needed only ~10us in)
            xT_src = xT_d.rearrange("(c p) s -> p c s", p=128)
            xs0 = p_stream.tile([128, KD, 512], BF16, tag="xs")
            wqk_sb = p_w.tile([128, KD, NM * 128], BF16, tag="wqk")
            wqk_src = wqk_d.rearrange("(c p) n -> p c n", p=128)
            # 4-chunk granularity balances serialized HWDGE descriptor-gen
            # against time-to-first-chunk
            KH = KD // 2
            nc.sync.dma_start(out=xs0[:, 0:KH, :], in_=xT_src[:, 0:KH, 0:512])
            nc.sync.dma_start(out=wqk_sb[:, 0:KH, :], in_=wqk_src[:, 0:KH, :])
            nc.sync.dma_start(out=xs0[:, KH:KD, :], in_=xT_src[:, KH:KD, 0:512])
            nc.sync.dma_start(out=wqk_sb[:, KH:KD, :], in_=wqk_src[:, KH:KD, :])

            misc_sb = p_w.tile([128, MISC_W], F32, tag="misc")
            nc.sync.dma_start(out=misc_sb[:, :], in_=misc_d[:, :])
            bqk_sb = misc_sb[:, 0:NM]
            bvbc_sb = misc_sb[:, NM:NM + NH * 64]
            mask_sb = p_w.tile([128, 128], BF16, tag="mask")
            nc.vector.tensor_copy(
                mask_sb[:, :], misc_sb[:, NM + NH * 64:NM + NH * 64 + 128]
            )
            ident_sb = p_w.tile([128, 128], BF16, tag="ident")
            nc.vector.tensor_copy(
                ident_sb[:, :], misc_sb[:, NM + NH * 64 + 128:MISC_W]
            )

            wv_sb = p_w.tile([128, KD, NH * 64], BF16, tag="wv")
            wv_src = wv_d.rearrange("(c p) n -> p c n", p=128)
            nc.sync.dma_start(out=wv_sb[:, :, :], in_=wv_src[:, :, :])

            xs1 = p_stream.tile([128, KD, 512], BF16, tag="xs", name="xs1")
            nc.sync.dma_start(out=xs1[:, :, :], in_=xT_src[:, :, 512:1024])

            # wproj is only needed by the projection fillers in the final
            # block (~90us in); load it behind everything the front needs
            wproj_sb = p_w.tile([128, NPAIR, D], BF16, tag="wproj")
            nc.sync.dma_start(out=wproj_sb[:, :, :], in_=wproj_d[:, :, :])

            qkT_sb = p_qkt.tile([128, NM, S], BF16, tag="qkt")
            v_aug = p_vaug.tile([128, NSK, NH, 65], BF16, tag="vaug")
            nc.vector.memset(v_aug[:, :, :, 64:65], 1.0)
            # attn2: pair-packed normalized attnT. partitions 0:64 head 2pp,
            # 64:128 head 2pp+1; slot pi = 2*j + pp
            attn2 = p_attn.tile([128, NPJ, 512], BF16, tag="attn")

            def load_xs(j):
                xs = p_stream.tile([128, KD, 512], BF16, tag="xs")
                nc.sync.dma_start(
                    out=xs[:, :, :], in_=xT_src[:, :, j * 512:(j + 1) * 512]
                )
                return xs

            def qk_move(j, mp, ps_qk):
                dst = qkT_sb[:, mp, j * 512:(j + 1) * 512]
                nc.vector.tensor_scalar_add(dst, ps_qk[:, :], bqk_sb[:, mp:mp + 1])

            def qk_steps(j, xs, mp):
                """Micro-steps (one matmul each) for one qk projection tile."""
                cell = {}

                def mm(k):
                    if k == 0:
                        cell["ps"] = p_misc.tile([128, 512], F32, tag="m", name="ps_qk")
                    nc.tensor.matmul(
                        cell["ps"][:, :],
                        lhsT=wqk_sb[:, k, mp * 128:(mp + 1) * 128],
                        rhs=xs[:, k, :],
                        start=(k == 0),
                        stop=(k == KD - 1),
                    )

                return [(True, lambda k=k: mm(k)) for k in range(KD)] + [
                    (False, lambda: qk_move(j, mp, cell["ps"]))
                ]

            def v_steps(j, xs, m):
                cell = {}

                def mm(k):
                    if k == 0:
                        cell["ps"] = p_misc.tile([128, NH * 64], F32, tag="m", name="ps_v")
                    nc.tensor.matmul(
                        cell["ps"][:, :],
                        lhsT=xs[:, k, (m % 4) * 128:(m % 4) * 128 + 128],
                        rhs=wv_sb[:, k, :],
                        start=(k == 0),
                        stop=(k == KD - 1),
                    )

                def mv():
                    nc.vector.tensor_add(
                        v_aug[:, m, :, 0:64],
                        cell["ps"][:, :].rearrange("p (h c) -> p h c", c=64),
                        bvbc_sb.rearrange("p (h c) -> p h c", c=64),
                    )

                return [(True, lambda k=k: mm(k)) for k in range(KD)] + [
                    (False, mv)
                ]

            def qkv_steps(j, xs):
                steps = []
                for mp in range(NM):
                    steps += qk_steps(j, xs, mp)
                for m in range(4 * j, 4 * j + 4):
                    steps += v_steps(j, xs, m)
                return steps

            def attention_block(j, fillers):
                # in the final block, hold back a few PE filler steps for the
                # ACT-paced last-head stretch
                reserve = [4 if j == SQB - 1 else 0]
                # per-group filler quota: qkv steps are 213ns, proj steps
                # (atomic 2-matmul chains) are 426ns; ACT outruns this loop's
                # PE work by ~350ns/group
                pump_n = 1 if j == SQB - 1 else 2

                def pump(n=1):
                    got = 0
                    while fillers and got < n:
                        if reserve[0] and sum(
                            1 for p, _ in fillers if p
                        ) <= reserve[0]:
                            return
                        is_pe, fn = fillers.pop(0)
                        fn()
                        if is_pe:
                            got += 1

                for h in range(NH):
                    qT = qkT_sb[64 * (h % 2):64 * (h % 2) + 64, h // 2, :]
                    kT = qkT_sb[64 * (h % 2):64 * (h % 2) + 64, NMQ + h // 2, :]
                    ps_avq = p_avq.tile([128, 4, 65], F32, tag="avq")
                    # the 4 AV chains share one PSUM bank; a per-chain matmul
                    # start=True would mark the whole 2KB bank pending-zero
                    # and wipe its neighbours' partials. Open the bank with
                    # one zero matmul covering every chain byte, then
                    # accumulate with start=False throughout.
                    nc.tensor.matmul(
                        ps_avq[:, :, :],
                        lhsT=zero_sb[:, :],
                        rhs=ones_sb[:, :],
                        start=True,
                        stop=False,
                        skip_group_check=True,
                    )
                    pp = h // 2
                    pi = 2 * j + pp
                    if h % 2 == 0:
                        asb_cur = p_asb.tile([128, 4, 128], BF16, tag="asb")
                    asb = asb_cur
                    npair = 2 * (j + 1)
                    tail = j == SQB - 1 and h == NH - 1

                    def noff(i):
                        # causal column truncation (bf16: no N>=256 minimum)
                        mb = i - 4 * j
                        return 0 if mb <= 0 else 128 * mb

                    def emit_scores(g):
                        # exact causal regions; the merged diag exp also reads
                        # stale PSUM outside them, which downstream AV never
                        # consumes (harmless garbage, skipped per subtile)
                        ps = p_s.tile([128, 2, 512], F32, tag="s")
                        for b in range(2):
                            i = 2 * g + b
                            no = noff(i)
                            nc.tensor.matmul(
                                ps[:, b, no:512],
                                lhsT=kT[:, i * 128:(i + 1) * 128],
                                rhs=qT[:, j * 512 + no:(j + 1) * 512],
                                start=True,
                                stop=True,
                            )
                        return ps

                    rcp = p_rcp.tile([128, 4, 1], F32, tag="rcp")

                    def norm_subtile(t):
                        """reciprocal of the ones-column sum + normalized
                        PSUM->SBUF copy for q-subtile t of this head."""
                        nc.vector.reciprocal(
                            rcp[:, t:t + 1, :], ps_avq[:, t:t + 1, 64:65]
                        )
                        nc.vector.tensor_scalar_mul(
                            asb[:, t, 64 * (h % 2):64 * (h % 2) + 64],
                            ps_avq[:, t, 0:64],
                            rcp[:, t, :],
                        )

                    def tail_subtile(t, psT):
                        """Last head: chain t stopped one b-step ago and its
                        normalize already ran on DVE. Transpose and close
                        projection m-tile 12+t (both 512-col halves)."""
                        nc.tensor.transpose(
                            psT[:, t, :], asb[:, t, :], ident_sb[:, :]
                        )
                        nc.vector.tensor_copy(
                            attn2[:, pi, 128 * t:128 * (t + 1)], psT[:, t, :]
                        )
                        y_sb = p_y.tile([128, 2, 512], BF16, tag="y",
                                        name="y_sb")
                        chs = []
                        for n in range(2):
                            ch = p_misc.tile([128, 512], F32, tag="m",
                                             name="tp_ps")
                            chs.append(ch)
                            nc.tensor.matmul(
                                ch[:, :],
                                lhsT=attn2[:, 2 * j, t * 128:(t + 1) * 128],
                                rhs=wproj_sb[:, 0, n * 512:(n + 1) * 512],
                                start=True,
                                stop=False,
                            )
                        for n in range(2):
                            nc.tensor.matmul(
                                chs[n][:, :],
                                lhsT=attn2[:, 2 * j + 1,
                                           t * 128:(t + 1) * 128],
                                rhs=wproj_sb[:, 1, n * 512:(n + 1) * 512],
                                start=False,
                                stop=True,
                            )
                        m = 4 * j + t
                        nc.scalar.copy(y_sb[:, 0, :], chs[0][:, :])
                        if t == 3:
                            # final m-tile: half DMAs overlap the second
                            # half's copy with the first half's transfer
                            nc.sync.dma_start(
                                out=y_d[m * 128:(m + 1) * 128, 0:512],
                                in_=y_sb[:, 0, :],
                            )
                            nc.vector.tensor_copy(y_sb[:, 1, :], chs[1][:, :])
                            nc.sync.dma_start(
                                out=y_d[m * 128:(m + 1) * 128, 512:1024],
                                in_=y_sb[:, 1, :],
                            )
                        else:
                            nc.vector.tensor_copy(y_sb[:, 1, :], chs[1][:, :])
                            nc.sync.dma_start(
                                out=y_d[m * 128:(m + 1) * 128, :],
                                in_=y_sb[:, :, :],
                            )

                    sc_next = emit_scores(0)
                    if tail:
                        psT = p_pst.tile([128, 4, 128], BF16, tag="pst")
                    for g in range(npair):
                        ps_sc = sc_next
                        # 1-deep software pipeline: next group's scores are
                        # emitted before this group's AV so PE runs them
                        # while ACT computes this group's exp
                        if g + 1 < npair:
                            sc_next = emit_scores(g + 1)
                        # ACT runs ~350ns/group longer than this loop's PE
                        # work; pump filler steps so PE never idles on exp
                        pump(pump_n)
                        exp_t = p_exp.tile([128, 2, 512], BF16, tag="exp")
                        if g == 2 * j:
                            # diag pair mb=0,1: single exp over both tiles
                            nc.scalar.activation(
                                exp_t[:, :, :], ps_sc[:, :, :], AFT.Exp, scale=SCALE
                            )
                            nc.vector.tensor_mul(
                                exp_t[:, 0, 0:128], exp_t[:, 0, 0:128], mask_sb[:, :]
                            )
                            nc.vector.tensor_mul(
                                exp_t[:, 1, 128:256], exp_t[:, 1, 128:256],
                                mask_sb[:, :],
                            )
                        elif g == 2 * j + 1:
                            # mb=2,3: exp the computed 256:512 of both tiles
                            nc.scalar.activation(
                                exp_t[:, :, 256:512],
                                ps_sc[:, :, 256:512],
                                AFT.Exp,
                                scale=SCALE,
                            )
                            nc.vector.tensor_mul(
                                exp_t[:, 0, 256:384], exp_t[:, 0, 256:384],
                                mask_sb[:, :],
                            )
                            nc.vector.tensor_mul(
                                exp_t[:, 1, 384:512], exp_t[:, 1, 384:512],
                                mask_sb[:, :],
                            )
                        else:
                            nc.scalar.activation(
                                exp_t[:, :, :], ps_sc[:, :, :], AFT.Exp, scale=SCALE
                            )
                        for b in range(2):
                            i = 2 * g + b
                            mb = i - 4 * j
                            for t in range(max(0, mb), 4):
                                nc.tensor.matmul(
                                    ps_avq[:, t, :],
                                    lhsT=exp_t[:, b, 128 * t:128 * (t + 1)],
                                    rhs=v_aug[:, i, h, :],
                                    start=False,
                                    stop=(i == 4 * j + t),
                                    skip_group_check=True,
                                )
                            if tail and mb >= 0:
                                # chain mb just stopped: normalize on DVE now
                                norm_subtile(mb)
                            if tail and mb >= 1:
                                # chain mb-1 normalized one b-step ago ->
                                # transpose + close its projection m-tile
                                tail_subtile(mb - 1, psT)
                    if tail:
                        # drain remaining fillers (their y DMAs must precede
                        # the final m-tile's), then close the last subtile
                        reserve[0] = 0
                        while fillers:
                            fillers.pop(0)[1]()
                        tail_subtile(3, psT)
                    elif h % 2 == 0:
                        for t in range(4):
                            norm_subtile(t)
                    else:
                        psT = p_pst.tile([128, 4, 128], BF16, tag="pst")
                        for t in range(4):
                            norm_subtile(t)
                        # fillers between the DVE normalizes and the PE
                        # transposes hide the normalize latency
                        pump(2)
                        for t in range(4):
                            nc.tensor.transpose(
                                psT[:, t, :], asb[:, t, :], ident_sb[:, :]
                            )
                        nc.vector.tensor_copy(attn2[:, pi, :], psT[:, :, :])
                    # drain PE filler work into the ACT-paced stretch,
                    # counting only PE (matmul) steps toward the quota
                    if h >= 1 and not tail:
                        npe = sum(1 for is_pe, _ in fillers if is_pe)
                        take = max(1, (npe - reserve[0]) // (6 * (NH - h)))
                        while fillers and take > 0:
                            if reserve[0] and sum(
                                1 for p, _ in fillers if p
                            ) <= reserve[0]:
                                break
                            is_pe, fn = fillers.pop(0)
                            fn()
                            if is_pe:
                                take -= 1
                while fillers:
                    fillers.pop(0)[1]()

            def proj_steps_m(j, m):
                """Micro-steps for one 128-row tile of the out-projection.
                Each 512-col chain (both pair accumulations) is one atomic
                step so a pump boundary never leaves a PSUM chain open while
                other code allocates from the same pool."""
                o = (m % 4) * 128
                cell = {}

                def mmv(n):
                    if n == 0:
                        cell["y"] = p_y.tile([128, 2, 512], BF16, tag="y",
                                             name="y_sb")
                    ps = p_misc.tile([128, 512], F32, tag="m", name="ps_y")
                    for pp in range(NPAIR):
                        nc.tensor.matmul(
                            ps[:, :],
                            lhsT=attn2[:, 2 * j + pp, o:o + 128],
                            rhs=wproj_sb[:, pp, n * 512:(n + 1) * 512],
                            start=(pp == 0),
                            stop=(pp == NPAIR - 1),
                        )
                    nc.vector.tensor_copy(cell["y"][:, n, :], ps[:, :])

                def out():
                    nc.sync.dma_start(
                        out=y_d[m * 128:(m + 1) * 128, :],
                        in_=cell["y"][:, :, :],
                    )

                steps = []
                for n in range(2):
                    steps.append((True, lambda n=n: mmv(n)))
                steps.append((False, out))
                return steps

            def proj_steps(j):
                steps = []
                for m in range(j * 4, j * 4 + 4):
                    steps += proj_steps_m(j, m)
                return steps

            # j=0 prologue. The startup is DMA-serial-bound: run all four qk
            # tiles chunk-half-major (4 concurrent PSUM chains, borrowing the
            # idle score pool) so PE tracks the half-chunk DMA cadence; then
            # v chunk-major the same way.
            junk(75)
            ps_qk0 = p_misc.tile([128, 512], F32, tag="m")
            ps_qk1 = p_misc.tile([128, 512], F32, tag="m")
            ps_qk23 = p_s.tile([128, 2, 512], F32, tag="s")
            chains = (ps_qk0[:, :], ps_qk1[:, :], ps_qk23[:, 0, :],
                      ps_qk23[:, 1, :])
            for kh in range(2):
                for k in range(kh * KH, (kh + 1) * KH):
                    for mp in range(NM):
                        nc.tensor.matmul(
                            chains[mp],
                            lhsT=wqk_sb[:, k, mp * 128:(mp + 1) * 128],
                            rhs=xs0[:, k, :],
                            start=(k == 0),
                            stop=(k == KD - 1),
                        )
            for mp in range(NM):
                qk_move(0, mp, chains[mp])
            ps_v01 = p_s.tile([128, 2, 512], F32, tag="s")
            vchains = (ps_v01[:, 0, 0:256], ps_v01[:, 1, 0:256], None, None)
            vcells = [None, None, None, None]
            for k in range(KD):
                for m in range(4):
                    if m < 2:
                        ps = vchains[m]
                    else:
                        if k == 0 and vcells[m] is None:
                            vcells[m] = p_misc.tile(
                                [128, NH * 64], F32, tag="m", name="ps_v"
                            )
                        ps = vcells[m][:, :]
                    nc.tensor.matmul(
                        ps,
                        lhsT=xs0[:, k, m * 128:m * 128 + 128],
                        rhs=wv_sb[:, k, :],
                        start=(k == 0),
                        stop=(k == KD - 1),
                    )
            for m in range(4):
                src = vchains[m] if m < 2 else vcells[m][:, :]
                nc.vector.tensor_add(
                    v_aug[:, m, :, 0:64],
                    src.rearrange("p (h c) -> p h c", c=64),
                    bvbc_sb.rearrange("p (h c) -> p h c", c=64),
                )
            xs_next = xs1 if SQB > 1 else None
            for j in range(SQB):
                fillers = []
                if j + 1 < SQB:
                    fillers += qkv_steps(j + 1, xs_next)
                    xs_after = load_xs(j + 2) if j + 2 < SQB else None
                else:
                    xs_after = None
                if j == SQB - 1:
                    for jp in range(SQB - 1):
                        fillers += proj_steps(jp)
                attention_block(j, fillers)
                xs_next = xs_after

    return nc


def make_mask4():
    p = np.arange(128)[:, None]
    f = np.arange(128)[None, :]
    return (f >= p).astype(np.float32).copy()  # [128, 128] lower-tri in T layout


def to_bf16(x):
    import ml_dtypes

    return np.asarray(x, dtype=np.float32).astype(ml_dtypes.bfloat16)


def make_in_maps(x, W_qkv, b_qkv, W_proj):
    """Per-core input dicts for the full-size problem (bf16 staged)."""
    mask4 = make_mask4()
    ident = np.eye(128, dtype=np.float32)
    in_maps = []
    for c in range(NCORES):
        b, q = c // 4, c % 4
        cq = slice(256 * q, 256 * q + 256)
        wqk = np.concatenate([W_qkv[:, cq], W_qkv[:, 1024:2048][:, cq]], axis=1)
        wv = W_qkv[:, 2048:3072][:, cq]
        bqk = np.concatenate([b_qkv[cq], b_qkv[1024:2048][cq]]).reshape(4, 128)
        bvbc = np.broadcast_to(b_qkv[2048:3072][cq], (128, 256))
        # packed misc input: [128, 4 bqk-cols | 256 bvbc | 128 mask | 128 id]
        misc = np.concatenate([bqk.T, bvbc, mask4, ident], axis=1).astype(
            np.float32
        )
        # pair-packed wproj: [p, pp, n] = W_proj[256*q + pp*128 + p, n]
        wproj = np.ascontiguousarray(
            W_proj[cq, :].reshape(2, 128, 1024).transpose(1, 0, 2)
        )
        in_maps.append(
            {
                "xT": np.ascontiguousarray(to_bf16(x[b].T)),
                "wqk": np.ascontiguousarray(to_bf16(wqk)),
                "wv": np.ascontiguousarray(to_bf16(wv)),
                "misc": np.ascontiguousarray(misc),
                "wproj": to_bf16(wproj),
            }
        )
    return in_maps


_NC_CACHE = {}


def _get_nc():
    if "nc" not in _NC_CACHE:
        _NC_CACHE["nc"] = build_nc()
    return _NC_CACHE["nc"]


def run_on_hw(x, W_qkv, b_qkv, W_proj, b_proj, trace=False, **trace_kw):
    from concourse.bass_utils import run_bass_kernel_spmd

    in_maps = make_in_maps(x, W_qkv, b_qkv, W_proj)
    res = run_bass_kernel_spmd(
        _get_nc(), in_maps, core_ids=list(range(NCORES)), trace=trace, **trace_kw
    )
    out = np.empty((B, S, D), dtype=np.float32)
    for b in range(B):
        acc = res.results[4 * b]["y"].astype(np.float32)
        for q in range(1, 4):
            acc = acc + res.results[4 * b + q]["y"].astype(np.float32)
        out[b] = acc + b_proj[None, :]
    return out, res


def kernel(x, W_qkv, b_qkv, W_proj, b_proj):
    x = np.asarray(x, dtype=np.float32)
    W_qkv = np.asarray(W_qkv, dtype=np.float32)
    b_qkv = np.asarray(b_qkv, dtype=np.float32)
    W_proj = np.asarray(W_proj, dtype=np.float32)
    b_proj = np.asarray(b_proj, dtype=np.float32)
    out, _ = run_on_hw(x, W_qkv, b_qkv, W_proj, b_proj, trace=False)
    return out


# revision 24
# speedup vs baseline: 1.0848x; 1.0837x over previous
"""Trainium2 Bass kernel v3 for causal multi-head self-attention.

Problem (hardcoded):
    x:      [2, 2048, 1024] f32
    W_qkv:  [1024, 3072] f32   (cols: [q | k | v], each 1024 = 16 heads x 64)
    b_qkv:  [3072] f32
    W_proj: [1024, 1024] f32
    b_proj: [1024] f32
    out:    [2, 2048, 1024] f32

Sharding over 8 NeuronCores: data parallel on batch (2) x tensor parallel on
heads (4 quads of 4 heads). Core c handles batch c//4, heads [4*(c%4), 4*(c%4)+4).
Host gather sums the 4 partial projections per batch and adds b_proj.

v3 changes vs v2:
  - AV matmul orientation flipped: out[q,hd] accumulators with exp weights as
    the stationary operand and the 65-col V(+ones) as the moving operand.
    Halves AV's PE column charge (the cost model charges moving columns only).
  - softmax normalize becomes a per-partition-scalar multiply (reciprocal of
    the ones-column sums + tensor_scalar_mul on the PSUM->SBUF copy); the
    DRAM-bounce broadcast, gpsimd multiplies, and scratch tensor are gone.
  - a cheap PE transpose (identity matmul, 128 cols/tile) restores the
    [hd, q] layout the out-projection consumes; pair-packed attn2 unchanged.
  - last block's tail is pipelined per 128-query subtile: each AV chain stops
    early, normalizes, transposes, and feeds its projection m-tile while the
    remaining chains still accumulate.
"""

import os
import sys

for _p in ("/opt/trn_rl_repo", "/root/.axon_site/_ro/trn_rl_repo"):
    if os.path.isdir(_p) and _p not in sys.path:
        sys.path.append(_p)

import numpy as np

import concourse.bass as bass
import concourse.mybir as mybir
import concourse.tile as tile
from concourse.alu_op_type import AluOpType

F32 = mybir.dt.float32
BF16 = mybir.dt.bfloat16
AFT = mybir.ActivationFunctionType

B, S, D, H, HD = 2, 2048, 1024, 16, 64
NCORES = 8
NH = 4  # heads per core
SCALE = 1.0 / 8.0  # 1/sqrt(64)


class SplitWaitTileContext(tile.TileContext):
    """This container's walrus rejects >1 sync wait per instruction
    ("Too many sync wait commands"). Split extra waits onto preceding
    same-engine NoOps before the final block lowering."""

    def _lower_ordered_insts(self, ordered):
        for bb_name, insts in list(ordered.items()):
            new = []
            for inst in insts:
                si = inst.sync_info
                if si is not None and si.on_wait and len(si.on_wait) > 1:
                    waits = list(si.on_wait)
                    for w in waits[:-1]:
                        nop = mybir.InstNoOp(
                            name=f"nopw-{self.nc.get_next_instruction_name()}"
                        )
                        nop.engine = inst.engine
                        nop.sync_info = mybir.SyncInfo(on_wait=[w], on_update=[])
                        new.append(nop)
                    inst.sync_info = mybir.SyncInfo(
                        on_wait=[waits[-1]], on_update=list(si.on_update or [])
                    )
                new.append(inst)
            ordered[bb_name] = new
        return super()._lower_ordered_insts(ordered)

    def _drain_and_barrier(self, tick_clock, wait_clock):
        from concourse.vector_clock import ScopedClock

        drain_inst = self.nc.sync.drain()
        wait_clock.add_sem_waits(
            drain_inst.ins, ScopedClock({None: tick_clock.global_clock})
        )
        si = drain_inst.ins.sync_info
        if si is not None and si.on_wait and len(si.on_wait) > 1:
            waits = list(si.on_wait)
            drain_inst.ins.sync_info = mybir.SyncInfo(
                on_wait=[waits[0]], on_update=list(si.on_update or [])
            )
            for w in waits[1:]:
                nop = self.nc.sync.nop(nofuse=True)
                nop.ins.sync_info = mybir.SyncInfo(on_wait=[w], on_update=[])

        self.nc.all_engine_barrier()
        assert self.sems is not None
        popped = self.nc._tile_sem_poison_stack.pop()
        assert popped is self._sem_poison
        self.nc.clear_and_free_semaphores(list(self.sems.allocated().values()))
        self.nc.all_engine_barrier()


def build_nc(S=S, D=D, NH=NH, dbg=False, reps=1):
    """Build the single-core SPMD program."""
    KD = D // 128             # 8 k-chunks of the D contraction
    NM = NH * 2 * 64 // 128   # 4 qk M-tiles (q chunks then k chunks)
    NMQ = NM // 2
    SQB = S // 512            # 4 sq blocks of 512
    NSK = S // 128            # 16 sk tiles of 128
    NPAIR = NH // 2           # 2 head pairs per block
    NPJ = SQB * NPAIR         # 8 pair-slots (pair index pi = 2*j + pp)
    MISC_W = NM + NH * 64 + 128 + 128  # bqk | bvbc | mask | identity

    nc = bass.Bass("TRN2", target_bir_lowering=False, debug=False)

    xT_d = nc.dram_tensor("xT", [D, S], BF16, kind="ExternalInput").ap()
    wqk_d = nc.dram_tensor("wqk", [D, NM * 128], BF16, kind="ExternalInput").ap()
    wv_d = nc.dram_tensor("wv", [D, NH * 64], BF16, kind="ExternalInput").ap()
    misc_d = nc.dram_tensor("misc", [128, MISC_W], F32, kind="ExternalInput").ap()
    # wproj pre-packed on host: [p, pp, n] = W_proj[core_base + pp*128 + p, n]
    wproj_d = nc.dram_tensor("wproj", [128, NPAIR, D], BF16, kind="ExternalInput").ap()
    y_d = nc.dram_tensor("y", [S, D], BF16, kind="ExternalOutput").ap()

    with SplitWaitTileContext(nc) as tc:
        with (
            nc.allow_low_precision(reason="bf16 feeds PE; fp32 accum in PSUM"),
            tc.tile_pool(name="stream", bufs=2) as p_stream,
            tc.tile_pool(name="attnp", bufs=1) as p_attn,
            tc.tile_pool(name="wpool", bufs=1) as p_w,
            tc.tile_pool(name="qkt", bufs=1) as p_qkt,
            tc.tile_pool(name="vaug", bufs=1) as p_vaug,
            tc.tile_pool(name="expp", bufs=8) as p_exp,
            tc.tile_pool(name="asb", bufs=2) as p_asb,
            tc.tile_pool(name="rcpp", bufs=2) as p_rcp,
            tc.tile_pool(name="ypool", bufs=8) as p_y,
            tc.tile_pool(name="pmisc", bufs=2, space="PSUM") as p_misc,
            tc.tile_pool(name="ps", bufs=2, space="PSUM") as p_s,
            tc.tile_pool(name="pavq", bufs=1, space="PSUM") as p_avq,
            tc.tile_pool(name="ppst", bufs=1, space="PSUM") as p_pst,
        ):
          for _rep in range(reps):
            # PE warmup: junk matmuls keep the systolic array ramped while the
            # input DMAs land
            ones_sb = p_w.tile([128, 260], BF16, tag="ones")
            nc.vector.memset(ones_sb[:, :], 1.0)
            zero_sb = p_w.tile([128, 128], BF16, tag="zero")
            nc.vector.memset(zero_sb[:, :], 0.0)
            # preload the exp table set in the startup window
            expwarm = p_w.tile([1, 1], F32, tag="expwarm")
            nc.scalar.activation(
                expwarm[:, :], ones_sb[0:1, 0:1], AFT.Exp, scale=SCALE
            )
            warm_ps = p_avq.tile([128, 4, 65], F32, tag="avq", name="warm_ps")

            def junk(n):
                # junk may leave stale pending-zero flags in the bank; the
                # per-head zero matmul re-marks and clears every chain byte,
                # so junk's footprint doesn't matter
                for _ in range(n):
                    nc.tensor.matmul(
                        warm_ps[0:64, 0, 0:64],
                        lhsT=ones_sb[:, 0:64],
                        rhs=ones_sb[:, 0:64],
                        start=True,
                        stop=True,
                    )

            junk(40)

            # input DMAs: transfers AND descriptor-gen (627ns HWDGE) serialize
            # device-wide, so the first-needed big transfers go first; tiny
            # bias/mask loads ride behind them (needed only ~10us in)
            xT_src = xT_d.rearrange("(c p) s -> p c s", p=128)
            xs0 = p_stream.tile([128, KD, 512], BF16, tag="xs")
            wqk_sb = p_w.tile([128, KD, NM * 128], BF16, tag="wqk")
            wqk_src = wqk_d.rearrange("(c p) n -> p c n", p=128)
            # 4-chunk granularity balances serialized HWDGE descriptor-gen
            # against time-to-first-chunk
            KH = KD // 2
            nc.sync.dma_start(out=xs0[:, 0:KH, :], in_=xT_src[:, 0:KH, 0:512])
            nc.sync.dma_start(out=wqk_sb[:, 0:KH, :], in_=wqk_src[:, 0:KH, :])
            nc.sync.dma_start(out=xs0[:, KH:KD, :], in_=xT_src[:, KH:KD, 0:512])
            nc.sync.dma_start(out=wqk_sb[:, KH:KD, :], in_=wqk_src[:, KH:KD, :])

            misc_sb = p_w.tile([128, MISC_W], F32, tag="misc")
            nc.sync.dma_start(out=misc_sb[:, :], in_=misc_d[:, :])
            bqk_sb = misc_sb[:, 0:NM]
            bvbc_sb = misc_sb[:, NM:NM + NH * 64]
            mask_sb = p_w.tile([128, 128], BF16, tag="mask")
            nc.vector.tensor_copy(
                mask_sb[:, :], misc_sb[:, NM + NH * 64:NM + NH * 64 + 128]
            )
            ident_sb = p_w.tile([128, 128], BF16, tag="ident")
            nc.vector.tensor_copy(
                ident_sb[:, :], misc_sb[:, NM + NH * 64 + 128:MISC_W]
            )

            wv_sb = p_w.tile([128, KD, NH * 64], BF16, tag="wv")
            wv_src = wv_d.rearrange("(c p) n -> p c n", p=128)
            nc.sync.dma_start(out=wv_sb[:, :, :], in_=wv_src[:, :, :])

            xs1 = p_stream.tile([128, KD, 512], BF16, tag="xs", name="xs1")
            nc.sync.dma_start(out=xs1[:, :, :], in_=xT_src[:, :, 512:1024])

            # wproj is only needed by the projection fillers in the final
            # block (~90us in); load it behind everything the front needs
            wproj_sb = p_w.tile([128, NPAIR, D], BF16, tag="wproj")
            nc.sync.dma_start(out=wproj_sb[:, :, :], in_=wproj_d[:, :, :])

            qkT_sb = p_qkt.tile([128, NM, S], BF16, tag="qkt")
            v_aug = p_vaug.tile([128, NSK, NH, 65], BF16, tag="vaug")
            nc.vector.memset(v_aug[:, :, :, 64:65], 1.0)
            # attn2: pair-packed normalized attnT. partitions 0:64 head 2pp,
            # 64:128 head 2pp+1; slot pi = 2*j + pp
            attn2 = p_attn.tile([128, NPJ, 512], BF16, tag="attn")

            def load_xs(j):
                xs = p_stream.tile([128, KD, 512], BF16, tag="xs")
                nc.sync.dma_start(
                    out=xs[:, :, :], in_=xT_src[:, :, j * 512:(j + 1) * 512]
                )
                return xs

            def qk_move(j, mp, ps_qk):
                dst = qkT_sb[:, mp, j * 512:(j + 1) * 512]
                nc.vector.tensor_scalar_add(dst, ps_qk[:, :], bqk_sb[:, mp:mp + 1])

            def qk_steps(j, xs, mp):
                """Micro-steps (one matmul each) for one qk projection tile."""
                cell = {}

                def mm(k):
                    if k == 0:
                        cell["ps"] = p_misc.tile([128, 512], F32, tag="m", name="ps_qk")
                    nc.tensor.matmul(
                        cell["ps"][:, :],
                        lhsT=wqk_sb[:, k, mp * 128:(mp + 1) * 128],
                        rhs=xs[:, k, :],
                        start=(k == 0),
                        stop=(k == KD - 1),
                    )

                return [(True, lambda k=k: mm(k)) for k in range(KD)] + [
                    (False, lambda: qk_move(j, mp, cell["ps"]))
                ]

            def v_steps(j, xs, m):
                cell = {}

                def mm(k):
                    if k == 0:
                        cell["ps"] = p_misc.tile([128, NH * 64], F32, tag="m", name="ps_v")
                    nc.tensor.matmul(
                        cell["ps"][:, :],
                        lhsT=xs[:, k, (m % 4) * 128:(m % 4) * 128 + 128],
                        rhs=wv_sb[:, k, :],
                        start=(k == 0),
                        stop=(k == KD - 1),
                    )

                def mv():
                    nc.vector.tensor_add(
                        v_aug[:, m, :, 0:64],
                        cell["ps"][:, :].rearrange("p (h c) -> p h c", c=64),
                        bvbc_sb.rearrange("p (h c) -> p h c", c=64),
                    )

                return [(True, lambda k=k: mm(k)) for k in range(KD)] + [
                    (False, mv)
                ]

            def qkv_steps(j, xs):
                steps = []
                for mp in range(NM):
                    steps += qk_steps(j, xs, mp)
                for m in range(4 * j, 4 * j + 4):
                    steps += v_steps(j, xs, m)
                return steps

            def attention_block(j, fillers):
                # in the final block, hold back a few PE filler steps for the
                # ACT-paced last-head stretch
                reserve = [4 if j == SQB - 1 else 0]
                # per-group filler quota: qkv steps are 213ns, proj steps
                # (atomic 2-matmul chains) are 426ns; ACT outruns this loop's
                # PE work by ~350ns/group
                pump_n = 1 if j == SQB - 1 else 2

                def pump(n=1):
                    got = 0
                    while fillers and got < n:
                        if reserve[0] and sum(
                            1 for p, _ in fillers if p
                        ) <= reserve[0]:
                            return
                        is_pe, fn = fillers.pop(0)
                        fn()
                        if is_pe:
                            got += 1

                for h in range(NH):
                    qT = qkT_sb[64 * (h % 2):64 * (h % 2) + 64, h // 2, :]
                    kT = qkT_sb[64 * (h % 2):64 * (h % 2) + 64, NMQ + h // 2, :]
                    ps_avq = p_avq.tile([128, 4, 65], F32, tag="avq")
                    # the 4 AV chains share one PSUM bank; a per-chain matmul
                    # start=True would mark the whole 2KB bank pending-zero
                    # and wipe its neighbours' partials. Open the bank with
                    # one zero matmul covering every chain byte, then
                    # accumulate with start=False throughout.
                    nc.tensor.matmul(
                        ps_avq[:, :, :],
                        lhsT=zero_sb[:, :],
                        rhs=ones_sb[:, :],
                        start=True,
                        stop=False,
                        skip_group_check=True,
                    )
                    pp = h // 2
                    pi = 2 * j + pp
                    if h % 2 == 0:
                        asb_cur = p_asb.tile([128, 4, 128], BF16, tag="asb")
                    asb = asb_cur
                    npair = 2 * (j + 1)
                    tail = j == SQB - 1 and h == NH - 1

                    def noff(i):
                        # causal column truncation (bf16: no N>=256 minimum)
                        mb = i - 4 * j
                        return 0 if mb <= 0 else 128 * mb

                    def emit_scores(g):
                        # exact causal regions; the merged diag exp also reads
                        # stale PSUM outside them, which downstream AV never
                        # consumes (harmless garbage, skipped per subtile)
                        ps = p_s.tile([128, 2, 512], F32, tag="s")
                        for b in range(2):
                            i = 2 * g + b
                            no = noff(i)
                            nc.tensor.matmul(
                                ps[:, b, no:512],
                                lhsT=kT[:, i * 128:(i + 1) * 128],
                                rhs=qT[:, j * 512 + no:(j + 1) * 512],
                                start=True,
                                stop=True,
                            )
                        return ps

                    rcp = p_rcp.tile([128, 4, 1], F32, tag="rcp")

                    def norm_subtile(t):
                        """reciprocal of the ones-column sum + normalized
                        PSUM->SBUF copy for q-subtile t of this head."""
                        nc.vector.reciprocal(
                            rcp[:, t:t + 1, :], ps_avq[:, t:t + 1, 64:65]
                        )
                        nc.vector.tensor_scalar_mul(
                            asb[:, t, 64 * (h % 2):64 * (h % 2) + 64],
                            ps_avq[:, t, 0:64],
                            rcp[:, t, :],
                        )

                    def tail_subtile(t, psT):
                        """Last head: chain t stopped one b-step ago and its
                        normalize already ran on DVE. Transpose and close
                        projection m-tile 12+t (both 512-col halves)."""
                        nc.tensor.transpose(
                            psT[:, t, :], asb[:, t, :], ident_sb[:, :]
                        )
                        nc.vector.tensor_copy(
                            attn2[:, pi, 128 * t:128 * (t + 1)], psT[:, t, :]
                        )
                        y_sb = p_y.tile([128, 2, 512], BF16, tag="y",
                                        name="y_sb")
                        chs = []
                        for n in range(2):
                            ch = p_misc.tile([128, 512], F32, tag="m",
                                             name="tp_ps")
                            chs.append(ch)
                            nc.tensor.matmul(
                                ch[:, :],
                                lhsT=attn2[:, 2 * j, t * 128:(t + 1) * 128],
                                rhs=wproj_sb[:, 0, n * 512:(n + 1) * 512],
                                start=True,
                                stop=False,
                            )
                        for n in range(2):
                            nc.tensor.matmul(
                                chs[n][:, :],
                                lhsT=attn2[:, 2 * j + 1,
                                           t * 128:(t + 1) * 128],
                                rhs=wproj_sb[:, 1, n * 512:(n + 1) * 512],
                                start=False,
                                stop=True,
                            )
                        m = 4 * j + t
                        nc.scalar.copy(y_sb[:, 0, :], chs[0][:, :])
                        if t == 3:
                            # final m-tile: half DMAs overlap the second
                            # half's copy with the first half's transfer
                            nc.sync.dma_start(
                                out=y_d[m * 128:(m + 1) * 128, 0:512],
                                in_=y_sb[:, 0, :],
                            )
                            nc.vector.tensor_copy(y_sb[:, 1, :], chs[1][:, :])
                            nc.sync.dma_start(
                                out=y_d[m * 128:(m + 1) * 128, 512:1024],
                                in_=y_sb[:, 1, :],
                            )
                        else:
                            nc.vector.tensor_copy(y_sb[:, 1, :], chs[1][:, :])
                            nc.sync.dma_start(
                                out=y_d[m * 128:(m + 1) * 128, :],
                                in_=y_sb[:, :, :],
                            )

                    sc_next = emit_scores(0)
                    if tail:
                        psT = p_pst.tile([128, 4, 128], BF16, tag="pst")
                    for g in range(npair):
                        ps_sc = sc_next
                        # 1-deep software pipeline: next group's scores are
                        # emitted before this group's AV so PE runs them
                        # while ACT computes this group's exp
                        if g + 1 < npair:
                            sc_next = emit_scores(g + 1)
                        # ACT runs ~350ns/group longer than this loop's PE
                        # work; pump filler steps so PE never idles on exp
                        pump(pump_n)
                        exp_t = p_exp.tile([128, 2, 512], BF16, tag="exp")
                        if g == 2 * j:
                            # diag pair mb=0,1: single exp over both tiles
                            nc.scalar.activation(
                                exp_t[:, :, :], ps_sc[:, :, :], AFT.Exp, scale=SCALE
                            )
                            nc.vector.tensor_mul(
                                exp_t[:, 0, 0:128], exp_t[:, 0, 0:128], mask_sb[:, :]
                            )
                            nc.vector.tensor_mul(
                                exp_t[:, 1, 128:256], exp_t[:, 1, 128:256],
                                mask_sb[:, :],
                            )
                        elif g == 2 * j + 1:
                            # mb=2,3: exp the computed 256:512 of both tiles
                            nc.scalar.activation(
                                exp_t[:, :, 256:512],
                                ps_sc[:, :, 256:512],
                                AFT.Exp,
                                scale=SCALE,
                            )
                            nc.vector.tensor_mul(
                                exp_t[:, 0, 256:384], exp_t[:, 0, 256:384],
                                mask_sb[:, :],
                            )
                            nc.vector.tensor_mul(
                                exp_t[:, 1, 384:512], exp_t[:, 1, 384:512],
                                mask_sb[:, :],
                            )
                        else:
                            nc.scalar.activation(
                                exp_t[:, :, :], ps_sc[:, :, :], AFT.Exp, scale=SCALE
                            )
                        for b in range(2):
                            i = 2 * g + b
                            mb = i - 4 * j
                            for t in range(max(0, mb), 4):
                                nc.tensor.matmul(
                                    ps_avq[:, t, :],
                                    lhsT=exp_t[:, b, 128 * t:128 * (t + 1)],
                                    rhs=v_aug[:, i, h, :],
                                    start=False,
                                    stop=(i == 4 * j + t),
                                    skip_group_check=True,
                                )
                            if tail and mb >= 0:
                                # chain mb just stopped: normalize on DVE now
                                norm_subtile(mb)
                            if tail and mb >= 1:
                                # chain mb-1 normalized one b-step ago ->
                                # transpose + close its projection m-tile
                                tail_subtile(mb - 1, psT)
                    if tail:
                        # drain remaining fillers (their y DMAs must precede
                        # the final m-tile's), then close the last subtile
                        reserve[0] = 0
                        while fillers:
                            fillers.pop(0)[1]()
                        tail_subtile(3, psT)
                    elif h % 2 == 0:
                        for t in range(4):
                            norm_subtile(t)
                    else:
                        psT = p_pst.tile([128, 4, 128], BF16, tag="pst")
                        for t in range(4):
                            norm_subtile(t)
                        # fillers between the DVE normalizes and the PE
                        # transposes hide the normalize latency
                        pump(2)
                        for t in range(4):
                            nc.tensor.transpose(
                                psT[:, t, :], asb[:, t, :], ident_sb[:, :]
                            )
                        nc.vector.tensor_copy(attn2[:, pi, :], psT[:, :, :])
                    # drain PE filler work into the ACT-paced stretch,
                    # counting only PE (matmul) steps toward the quota
                    if h >= 1 and not tail:
                        npe = sum(1 for is_pe, _ in fillers if is_pe)
                        take = max(1, (npe - reserve[0]) // (6 * (NH - h)))
                        while fillers and take > 0:
                            if reserve[0] and sum(
                                1 for p, _ in fillers if p
                            ) <= reserve[0]:
                                break
                            is_pe, fn = fillers.pop(0)
                            fn()
                            if is_pe:
                                take -= 1
                while fillers:
                    fillers.pop(0)[1]()

            def proj_steps_m(j, m):
                """Micro-steps for one 128-row tile of the out-projection.
                Each 512-col chain (both pair accumulations) is one atomic
                step so a pump boundary never leaves a PSUM chain open while
                other code allocates from the same pool."""
                o = (m % 4) * 128
                cell = {}

                def mmv(n):
                    if n == 0:
                        cell["y"] = p_y.tile([128, 2, 512], BF16, tag="y",
                                             name="y_sb")
                    ps = p_misc.tile([128, 512], F32, tag="m", name="ps_y")
                    for pp in range(NPAIR):
                        nc.tensor.matmul(
                            ps[:, :],
                            lhsT=attn2[:, 2 * j + pp, o:o + 128],
                            rhs=wproj_sb[:, pp, n * 512:(n + 1) * 512],
                            start=(pp == 0),
                            stop=(pp == NPAIR - 1),
                        )
                    nc.vector.tensor_copy(cell["y"][:, n, :], ps[:, :])

                def out():
                    nc.sync.dma_start(
                        out=y_d[m * 128:(m + 1) * 128, :],
                        in_=cell["y"][:, :, :],
                    )

                steps = []
                for n in range(2):
                    steps.append((True, lambda n=n: mmv(n)))
                steps.append((False, out))
                return steps

            def proj_steps(j):
                steps = []
                for m in range(j * 4, j * 4 + 4):
                    steps += proj_steps_m(j, m)
                return steps

            # j=0 prologue. The startup is DMA-serial-bound: run all four qk
            # tiles chunk-half-major (4 concurrent PSUM chains, borrowing the
            # idle score pool) so PE tracks the half-chunk DMA cadence; then
            # v chunk-major the same way.
            junk(75)
            ps_qk0 = p_misc.tile([128, 512], F32, tag="m")
            ps_qk1 = p_misc.tile([128, 512], F32, tag="m")
            ps_qk23 = p_s.tile([128, 2, 512], F32, tag="s")
            chains = (ps_qk0[:, :], ps_qk1[:, :], ps_qk23[:, 0, :],
                      ps_qk23[:, 1, :])
            for kh in range(2):
                for k in range(kh * KH, (kh + 1) * KH):
                    for mp in range(NM):
                        nc.tensor.matmul(
                            chains[mp],
                            lhsT=wqk_sb[:, k, mp * 128:(mp + 1) * 128],
                            rhs=xs0[:, k, :],
                            start=(k == 0),
                            stop=(k == KD - 1),
                        )
            for mp in range(NM):
                qk_move(0, mp, chains[mp])
            ps_v01 = p_s.tile([128, 2, 512], F32, tag="s")
            vchains = (ps_v01[:, 0, 0:256], ps_v01[:, 1, 0:256], None, None)
            vcells = [None, None, None, None]
            for k in range(KD):
                for m in range(4):
                    if m < 2:
                        ps = vchains[m]
                    else:
                        if k == 0 and vcells[m] is None:
                            vcells[m] = p_misc.tile(
                                [128, NH * 64], F32, tag="m", name="ps_v"
                            )
                        ps = vcells[m][:, :]
                    nc.tensor.matmul(
                        ps,
                        lhsT=xs0[:, k, m * 128:m * 128 + 128],
                        rhs=wv_sb[:, k, :],
                        start=(k == 0),
                        stop=(k == KD - 1),
                    )
            for m in range(4):
                src = vchains[m] if m < 2 else vcells[m][:, :]
                nc.vector.tensor_add(
                    v_aug[:, m, :, 0:64],
                    src.rearrange("p (h c) -> p h c", c=64),
                    bvbc_sb.rearrange("p (h c) -> p h c", c=64),
                )
            xs_next = xs1 if SQB > 1 else None
            for j in range(SQB):
                fillers = []
                if j + 1 < SQB:
                    fillers += qkv_steps(j + 1, xs_next)
                    xs_after = load_xs(j + 2) if j + 2 < SQB else None
                else:
                    xs_after = None
                if j == SQB - 1:
                    for jp in range(SQB - 1):
                        fillers += proj_steps(jp)
                attention_block(j, fillers)
                xs_next = xs_after

    return nc


def make_mask4():
    p = np.arange(128)[:, None]
    f = np.arange(128)[None, :]
    return (f >= p).astype(np.float32).copy()  # [128, 128] lower-tri in T layout


def to_bf16(x):
    import ml_dtypes

    return np.asarray(x, dtype=np.float32).astype(ml_dtypes.bfloat16)


def make_in_maps(x, W_qkv, b_qkv, W_proj):
    """Per-core input dicts for the full-size problem (bf16 staged)."""
    mask4 = make_mask4()
    ident = np.eye(128, dtype=np.float32)
    in_maps = []
    for c in range(NCORES):
        b, q = c // 4, c % 4
        cq = slice(256 * q, 256 * q + 256)
        wqk = np.concatenate([W_qkv[:, cq], W_qkv[:, 1024:2048][:, cq]], axis=1)
        wv = W_qkv[:, 2048:3072][:, cq]
        bqk = np.concatenate([b_qkv[cq], b_qkv[1024:2048][cq]]).reshape(4, 128)
        bvbc = np.broadcast_to(b_qkv[2048:3072][cq], (128, 256))
        # packed misc input: [128, 4 bqk-cols | 256 bvbc | 128 mask | 128 id]
        misc = np.concatenate([bqk.T, bvbc, mask4, ident], axis=1).astype(
            np.float32
        )
        # pair-packed wproj: [p, pp, n] = W_proj[256*q + pp*128 + p, n]
        wproj = np.ascontiguousarray(
            W_proj[cq, :].reshape(2, 128, 1024).transpose(1, 0, 2)
        )
        in_maps.append(
            {
                "xT": np.ascontiguousarray(to_bf16(x[b].T)),
                "wqk": np.ascontiguousarray(to_bf16(wqk)),
                "wv": np.ascontiguousarray(to_bf16(wv)),
                "misc": np.ascontiguousarray(misc),
                "wproj": to_bf16(wproj),
            }
        )
    return in_maps


_NC_CACHE = {}


def _get_nc():
    if "nc" not in _NC_CACHE:
        _NC_CACHE["nc"] = build_nc()
    return _NC_CACHE["nc"]


def run_on_hw(x, W_qkv, b_qkv, W_proj, b_proj, trace=False, **trace_kw):
    from concourse.bass_utils import run_bass_kernel_spmd

    in_maps = make_in_maps(x, W_qkv, b_qkv, W_proj)
    res = run_bass_kernel_spmd(
        _get_nc(), in_maps, core_ids=list(range(NCORES)), trace=trace, **trace_kw
    )
    out = np.empty((B, S, D), dtype=np.float32)
    for b in range(B):
        acc = res.results[4 * b]["y"].astype(np.float32)
        for q in range(1, 4):
            acc = acc + res.results[4 * b + q]["y"].astype(np.float32)
        out[b] = acc + b_proj[None, :]
    return out, res


def kernel(x, W_qkv, b_qkv, W_proj, b_proj):
    x = np.asarray(x, dtype=np.float32)
    W_qkv = np.asarray(W_qkv, dtype=np.float32)
    b_qkv = np.asarray(b_qkv, dtype=np.float32)
    W_proj = np.asarray(W_proj, dtype=np.float32)
    b_proj = np.asarray(b_proj, dtype=np.float32)
    out, _ = run_on_hw(x, W_qkv, b_qkv, W_proj, b_proj, trace=False)
    return out


# revision 28
# speedup vs baseline: 1.0916x; 1.0062x over previous
"""Trainium2 Bass kernel v3 for causal multi-head self-attention.

Problem (hardcoded):
    x:      [2, 2048, 1024] f32
    W_qkv:  [1024, 3072] f32   (cols: [q | k | v], each 1024 = 16 heads x 64)
    b_qkv:  [3072] f32
    W_proj: [1024, 1024] f32
    b_proj: [1024] f32
    out:    [2, 2048, 1024] f32

Sharding over 8 NeuronCores: data parallel on batch (2) x tensor parallel on
heads (4 quads of 4 heads). Core c handles batch c//4, heads [4*(c%4), 4*(c%4)+4).
Host gather sums the 4 partial projections per batch and adds b_proj.

v3 changes vs v2:
  - AV matmul orientation flipped: out[q,hd] accumulators with exp weights as
    the stationary operand and the 65-col V(+ones) as the moving operand.
    Halves AV's PE column charge (the cost model charges moving columns only).
  - softmax normalize becomes a per-partition-scalar multiply (reciprocal of
    the ones-column sums + tensor_scalar_mul on the PSUM->SBUF copy); the
    DRAM-bounce broadcast, gpsimd multiplies, and scratch tensor are gone.
  - a cheap PE transpose (identity matmul, 128 cols/tile) restores the
    [hd, q] layout the out-projection consumes; pair-packed attn2 unchanged.
  - last block's tail is pipelined per 128-query subtile: each AV chain stops
    early, normalizes, transposes, and feeds its projection m-tile while the
    remaining chains still accumulate.
"""

import os
import sys

for _p in ("/opt/trn_rl_repo", "/root/.axon_site/_ro/trn_rl_repo"):
    if os.path.isdir(_p) and _p not in sys.path:
        sys.path.append(_p)

import numpy as np

import concourse.bass as bass
import concourse.mybir as mybir
import concourse.tile as tile
from concourse.alu_op_type import AluOpType

F32 = mybir.dt.float32
BF16 = mybir.dt.bfloat16
AFT = mybir.ActivationFunctionType

B, S, D, H, HD = 2, 2048, 1024, 16, 64
NCORES = 8
NH = 4  # heads per core
SCALE = 1.0 / 8.0  # 1/sqrt(64)


class SplitWaitTileContext(tile.TileContext):
    """This container's walrus rejects >1 sync wait per instruction
    ("Too many sync wait commands"). Split extra waits onto preceding
    same-engine NoOps before the final block lowering."""

    def _lower_ordered_insts(self, ordered):
        for bb_name, insts in list(ordered.items()):
            new = []
            for inst in insts:
                si = inst.sync_info
                if si is not None and si.on_wait and len(si.on_wait) > 1:
                    waits = list(si.on_wait)
                    for w in waits[:-1]:
                        nop = mybir.InstNoOp(
                            name=f"nopw-{self.nc.get_next_instruction_name()}"
                        )
                        nop.engine = inst.engine
                        nop.sync_info = mybir.SyncInfo(on_wait=[w], on_update=[])
                        new.append(nop)
                    inst.sync_info = mybir.SyncInfo(
                        on_wait=[waits[-1]], on_update=list(si.on_update or [])
                    )
                new.append(inst)
            ordered[bb_name] = new
        return super()._lower_ordered_insts(ordered)

    def _drain_and_barrier(self, tick_clock, wait_clock):
        from concourse.vector_clock import ScopedClock

        drain_inst = self.nc.sync.drain()
        wait_clock.add_sem_waits(
            drain_inst.ins, ScopedClock({None: tick_clock.global_clock})
        )
        si = drain_inst.ins.sync_info
        if si is not None and si.on_wait and len(si.on_wait) > 1:
            waits = list(si.on_wait)
            drain_inst.ins.sync_info = mybir.SyncInfo(
                on_wait=[waits[0]], on_update=list(si.on_update or [])
            )
            for w in waits[1:]:
                nop = self.nc.sync.nop(nofuse=True)
                nop.ins.sync_info = mybir.SyncInfo(on_wait=[w], on_update=[])

        self.nc.all_engine_barrier()
        assert self.sems is not None
        popped = self.nc._tile_sem_poison_stack.pop()
        assert popped is self._sem_poison
        self.nc.clear_and_free_semaphores(list(self.sems.allocated().values()))
        self.nc.all_engine_barrier()


def build_nc(S=S, D=D, NH=NH, dbg=False, reps=1):
    """Build the single-core SPMD program."""
    KD = D // 128             # 8 k-chunks of the D contraction
    NM = NH * 2 * 64 // 128   # 4 qk M-tiles (q chunks then k chunks)
    NMQ = NM // 2
    SQB = S // 512            # 4 sq blocks of 512
    NSK = S // 128            # 16 sk tiles of 128
    NPAIR = NH // 2           # 2 head pairs per block
    NPJ = SQB * NPAIR         # 8 pair-slots (pair index pi = 2*j + pp)
    MISC_W = NM + NH * 64 + 128 + 128  # bqk | bvbc | mask | identity

    nc = bass.Bass("TRN2", target_bir_lowering=False, debug=False)

    xT_d = nc.dram_tensor("xT", [D, S], BF16, kind="ExternalInput").ap()
    wqk_d = nc.dram_tensor("wqk", [D, NM * 128], BF16, kind="ExternalInput").ap()
    wv_d = nc.dram_tensor("wv", [D, NH * 64], BF16, kind="ExternalInput").ap()
    misc_d = nc.dram_tensor("misc", [128, MISC_W], F32, kind="ExternalInput").ap()
    # wproj pre-packed on host: [p, pp, n] = W_proj[core_base + pp*128 + p, n]
    wproj_d = nc.dram_tensor("wproj", [128, NPAIR, D], BF16, kind="ExternalInput").ap()
    y_d = nc.dram_tensor("y", [S, D], BF16, kind="ExternalOutput").ap()

    with SplitWaitTileContext(nc) as tc:
        with (
            nc.allow_low_precision(reason="bf16 feeds PE; fp32 accum in PSUM"),
            tc.tile_pool(name="stream", bufs=2) as p_stream,
            tc.tile_pool(name="attnp", bufs=1) as p_attn,
            tc.tile_pool(name="wpool", bufs=1) as p_w,
            tc.tile_pool(name="qkt", bufs=1) as p_qkt,
            tc.tile_pool(name="vaug", bufs=1) as p_vaug,
            tc.tile_pool(name="expp", bufs=8) as p_exp,
            tc.tile_pool(name="asb", bufs=2) as p_asb,
            tc.tile_pool(name="rcpp", bufs=2) as p_rcp,
            tc.tile_pool(name="ypool", bufs=8) as p_y,
            tc.tile_pool(name="pmisc", bufs=2, space="PSUM") as p_misc,
            tc.tile_pool(name="ps", bufs=2, space="PSUM") as p_s,
            tc.tile_pool(name="pavq", bufs=1, space="PSUM") as p_avq,
            tc.tile_pool(name="ppst", bufs=1, space="PSUM") as p_pst,
        ):
          for _rep in range(reps):
            # PE warmup: junk matmuls keep the systolic array ramped while the
            # input DMAs land
            ones_sb = p_w.tile([128, 260], BF16, tag="ones")
            # junk only needs the first 64 cols; memset those first so the
            # PE warmup starts as early as possible
            nc.vector.memset(ones_sb[:, 0:64], 1.0)
            nc.vector.memset(ones_sb[:, 64:260], 1.0)
            zero_sb = p_w.tile([128, 128], BF16, tag="zero")
            nc.vector.memset(zero_sb[:, :], 0.0)
            # preload the exp table set in the startup window
            expwarm = p_w.tile([1, 1], F32, tag="expwarm")
            nc.scalar.activation(
                expwarm[:, :], ones_sb[0:1, 0:1], AFT.Exp, scale=SCALE
            )
            warm_ps = p_avq.tile([128, 4, 65], F32, tag="avq", name="warm_ps")

            def junk(n):
                # junk may leave stale pending-zero flags in the bank; the
                # per-head zero matmul re-marks and clears every chain byte,
                # so junk's footprint doesn't matter
                for _ in range(n):
                    nc.tensor.matmul(
                        warm_ps[0:64, 0, 0:64],
                        lhsT=ones_sb[:, 0:64],
                        rhs=ones_sb[:, 0:64],
                        start=True,
                        stop=True,
                    )

            junk(40)

            # input DMAs: transfers AND descriptor-gen (627ns HWDGE) serialize
            # device-wide, so the first-needed big transfers go first; tiny
            # bias/mask loads ride behind them (needed only ~10us in)
            xT_src = xT_d.rearrange("(c p) s -> p c s", p=128)
            xs0 = p_stream.tile([128, KD, 512], BF16, tag="xs")
            wqk_sb = p_w.tile([128, KD, NM * 128], BF16, tag="wqk")
            wqk_src = wqk_d.rearrange("(c p) n -> p c n", p=128)
            # 4-chunk granularity balances serialized HWDGE descriptor-gen
            # against time-to-first-chunk
            KH = KD // 2
            nc.sync.dma_start(out=xs0[:, 0:KH, :], in_=xT_src[:, 0:KH, 0:512])
            nc.sync.dma_start(out=wqk_sb[:, 0:KH, :], in_=wqk_src[:, 0:KH, :])
            nc.sync.dma_start(out=xs0[:, KH:KD, :], in_=xT_src[:, KH:KD, 0:512])
            nc.sync.dma_start(out=wqk_sb[:, KH:KD, :], in_=wqk_src[:, KH:KD, :])

            misc_sb = p_w.tile([128, MISC_W], F32, tag="misc")
            nc.sync.dma_start(out=misc_sb[:, :], in_=misc_d[:, :])
            bqk_sb = misc_sb[:, 0:NM]
            bvbc_sb = misc_sb[:, NM:NM + NH * 64]
            mask_sb = p_w.tile([128, 128], BF16, tag="mask")
            nc.vector.tensor_copy(
                mask_sb[:, :], misc_sb[:, NM + NH * 64:NM + NH * 64 + 128]
            )
            ident_sb = p_w.tile([128, 128], BF16, tag="ident")
            nc.vector.tensor_copy(
                ident_sb[:, :], misc_sb[:, NM + NH * 64 + 128:MISC_W]
            )

            wv_sb = p_w.tile([128, KD, NH * 64], BF16, tag="wv")
            wv_src = wv_d.rearrange("(c p) n -> p c n", p=128)
            nc.sync.dma_start(out=wv_sb[:, :, :], in_=wv_src[:, :, :])

            xs1 = p_stream.tile([128, KD, 512], BF16, tag="xs", name="xs1")
            nc.sync.dma_start(out=xs1[:, :, :], in_=xT_src[:, :, 512:1024])

            # wproj is only needed by the projection fillers in the final
            # block (~90us in); load it behind everything the front needs
            wproj_sb = p_w.tile([128, NPAIR, D], BF16, tag="wproj")
            nc.sync.dma_start(out=wproj_sb[:, :, :], in_=wproj_d[:, :, :])

            qkT_sb = p_qkt.tile([128, NM, S], BF16, tag="qkt")
            v_aug = p_vaug.tile([128, NSK, NH, 65], BF16, tag="vaug")
            nc.vector.memset(v_aug[:, :, :, 64:65], 1.0)
            # attn2: pair-packed normalized attnT. partitions 0:64 head 2pp,
            # 64:128 head 2pp+1; slot pi = 2*j + pp
            attn2 = p_attn.tile([128, NPJ, 512], BF16, tag="attn")

            def load_xs(j):
                xs = p_stream.tile([128, KD, 512], BF16, tag="xs")
                nc.sync.dma_start(
                    out=xs[:, :, :], in_=xT_src[:, :, j * 512:(j + 1) * 512]
                )
                return xs

            def qk_move(j, mp, ps_qk):
                dst = qkT_sb[:, mp, j * 512:(j + 1) * 512]
                nc.vector.tensor_scalar_add(dst, ps_qk[:, :], bqk_sb[:, mp:mp + 1])

            def qk_steps(j, xs, mp):
                """Micro-steps (one matmul each) for one qk projection tile."""
                cell = {}

                def mm(k):
                    if k == 0:
                        cell["ps"] = p_misc.tile([128, 512], F32, tag="m", name="ps_qk")
                    nc.tensor.matmul(
                        cell["ps"][:, :],
                        lhsT=wqk_sb[:, k, mp * 128:(mp + 1) * 128],
                        rhs=xs[:, k, :],
                        start=(k == 0),
                        stop=(k == KD - 1),
                    )

                return [(True, lambda k=k: mm(k)) for k in range(KD)] + [
                    (False, lambda: qk_move(j, mp, cell["ps"]))
                ]

            def v_steps(j, xs, m):
                cell = {}

                def mm(k):
                    if k == 0:
                        cell["ps"] = p_misc.tile([128, NH * 64], F32, tag="m", name="ps_v")
                    nc.tensor.matmul(
                        cell["ps"][:, :],
                        lhsT=xs[:, k, (m % 4) * 128:(m % 4) * 128 + 128],
                        rhs=wv_sb[:, k, :],
                        start=(k == 0),
                        stop=(k == KD - 1),
                    )

                def mv():
                    nc.vector.tensor_add(
                        v_aug[:, m, :, 0:64],
                        cell["ps"][:, :].rearrange("p (h c) -> p h c", c=64),
                        bvbc_sb.rearrange("p (h c) -> p h c", c=64),
                    )

                return [(True, lambda k=k: mm(k)) for k in range(KD)] + [
                    (False, mv)
                ]

            def qkv_steps(j, xs):
                steps = []
                for mp in range(NM):
                    steps += qk_steps(j, xs, mp)
                for m in range(4 * j, 4 * j + 4):
                    steps += v_steps(j, xs, m)
                return steps

            def attention_block(j, fillers):
                # the final block's ACT chain is saturated; PE idle there is
                # free, so spend fillers early and keep the tail clean
                reserve = [0]
                pump_n = 2

                def pump(n=1):
                    got = 0
                    while fillers and got < n:
                        if reserve[0] and sum(
                            1 for p, _ in fillers if p
                        ) <= reserve[0]:
                            return
                        is_pe, fn = fillers.pop(0)
                        fn()
                        if is_pe:
                            got += 1

                def emit_scores_h(hh, g):
                    # exact causal regions; the merged diag exp also reads
                    # stale PSUM outside them, which downstream AV never
                    # consumes (harmless garbage, skipped per subtile)
                    qTh = qkT_sb[64 * (hh % 2):64 * (hh % 2) + 64, hh // 2, :]
                    kTh = qkT_sb[64 * (hh % 2):64 * (hh % 2) + 64,
                                 NMQ + hh // 2, :]
                    ps = p_s.tile([128, 2, 512], F32, tag="s")
                    for b in range(2):
                        i = 2 * g + b
                        mb = i - 4 * j
                        no = 0 if mb <= 0 else 128 * mb
                        nc.tensor.matmul(
                            ps[:, b, no:512],
                            lhsT=kTh[:, i * 128:(i + 1) * 128],
                            rhs=qTh[:, j * 512 + no:(j + 1) * 512],
                            start=True,
                            stop=True,
                        )
                    return ps

                carry = None
                for h in range(NH):
                    ps_avq = p_avq.tile([128, 4, 65], F32, tag="avq")
                    # the 4 AV chains share one PSUM bank; a per-chain matmul
                    # start=True would mark the whole 2KB bank pending-zero
                    # and wipe its neighbours' partials. Open the bank with
                    # one zero matmul covering every chain byte, then
                    # accumulate with start=False throughout.
                    nc.tensor.matmul(
                        ps_avq[:, :, :],
                        lhsT=zero_sb[:, :],
                        rhs=ones_sb[:, :],
                        start=True,
                        stop=False,
                        skip_group_check=True,
                    )
                    pp = h // 2
                    pi = 2 * j + pp
                    if h % 2 == 0:
                        asb_cur = p_asb.tile([128, 4, 128], BF16, tag="asb")
                    asb = asb_cur
                    npair = 2 * (j + 1)
                    tail = j == SQB - 1 and h == NH - 1

                    rcp = p_rcp.tile([128, 4, 1], F32, tag="rcp")

                    def norm_subtile(t):
                        """reciprocal of the ones-column sum + normalized
                        PSUM->SBUF copy for q-subtile t of this head."""
                        nc.vector.reciprocal(
                            rcp[:, t:t + 1, :], ps_avq[:, t:t + 1, 64:65]
                        )
                        nc.vector.tensor_scalar_mul(
                            asb[:, t, 64 * (h % 2):64 * (h % 2) + 64],
                            ps_avq[:, t, 0:64],
                            rcp[:, t, :],
                        )

                    def tail_subtile(t, psT):
                        """Last head: chain t stopped one b-step ago and its
                        normalize already ran on DVE. Transpose and close
                        projection m-tile 12+t (both 512-col halves)."""
                        nc.tensor.transpose(
                            psT[:, t, :], asb[:, t, :], ident_sb[:, :]
                        )
                        nc.vector.tensor_copy(
                            attn2[:, pi, 128 * t:128 * (t + 1)], psT[:, t, :]
                        )
                        y_sb = p_y.tile([128, 2, 512], BF16, tag="y",
                                        name="y_sb")
                        chs = []
                        for n in range(2):
                            ch = p_misc.tile([128, 512], F32, tag="m",
                                             name="tp_ps")
                            chs.append(ch)
                            nc.tensor.matmul(
                                ch[:, :],
                                lhsT=attn2[:, 2 * j, t * 128:(t + 1) * 128],
                                rhs=wproj_sb[:, 0, n * 512:(n + 1) * 512],
                                start=True,
                                stop=False,
                            )
                        for n in range(2):
                            nc.tensor.matmul(
                                chs[n][:, :],
                                lhsT=attn2[:, 2 * j + 1,
                                           t * 128:(t + 1) * 128],
                                rhs=wproj_sb[:, 1, n * 512:(n + 1) * 512],
                                start=False,
                                stop=True,
                            )
                        m = 4 * j + t
                        nc.scalar.copy(y_sb[:, 0, :], chs[0][:, :])
                        if t == 3:
                            # final m-tile: half DMAs overlap the second
                            # half's copy with the first half's transfer
                            nc.sync.dma_start(
                                out=y_d[m * 128:(m + 1) * 128, 0:512],
                                in_=y_sb[:, 0, :],
                            )
                            nc.vector.tensor_copy(y_sb[:, 1, :], chs[1][:, :])
                            nc.sync.dma_start(
                                out=y_d[m * 128:(m + 1) * 128, 512:1024],
                                in_=y_sb[:, 1, :],
                            )
                        else:
                            nc.vector.tensor_copy(y_sb[:, 1, :], chs[1][:, :])
                            nc.sync.dma_start(
                                out=y_d[m * 128:(m + 1) * 128, :],
                                in_=y_sb[:, :, :],
                            )

                    sc_next = carry if carry is not None else emit_scores_h(h, 0)
                    carry = None
                    if tail:
                        psT = p_pst.tile([128, 4, 128], BF16, tag="pst")
                    for g in range(npair):
                        ps_sc = sc_next
                        # 1-deep software pipeline: next group's scores are
                        # emitted before this group's AV so PE runs them
                        # while ACT computes this group's exp. The pipeline
                        # carries across heads (same block) so ACT never
                        # bubbles at a head boundary.
                        if g + 1 < npair:
                            sc_next = emit_scores_h(h, g + 1)
                        elif h + 1 < NH:
                            carry = emit_scores_h(h + 1, 0)
                        # ACT runs ~350ns/group longer than this loop's PE
                        # work; pump filler steps so PE never idles on exp
                        pump(pump_n)
                        exp_t = p_exp.tile([128, 2, 512], BF16, tag="exp")
                        if g == 2 * j:
                            # diag pair mb=0,1: single exp over both tiles
                            nc.scalar.activation(
                                exp_t[:, :, :], ps_sc[:, :, :], AFT.Exp, scale=SCALE
                            )
                            nc.vector.tensor_mul(
                                exp_t[:, 0, 0:128], exp_t[:, 0, 0:128], mask_sb[:, :]
                            )
                            nc.vector.tensor_mul(
                                exp_t[:, 1, 128:256], exp_t[:, 1, 128:256],
                                mask_sb[:, :],
                            )
                        elif g == 2 * j + 1:
                            # mb=2,3: exp the computed 256:512 of both tiles
                            nc.scalar.activation(
                                exp_t[:, :, 256:512],
                                ps_sc[:, :, 256:512],
                                AFT.Exp,
                                scale=SCALE,
                            )
                            nc.vector.tensor_mul(
                                exp_t[:, 0, 256:384], exp_t[:, 0, 256:384],
                                mask_sb[:, :],
                            )
                            nc.vector.tensor_mul(
                                exp_t[:, 1, 384:512], exp_t[:, 1, 384:512],
                                mask_sb[:, :],
                            )
                        else:
                            nc.scalar.activation(
                                exp_t[:, :, :], ps_sc[:, :, :], AFT.Exp, scale=SCALE
                            )
                        for b in range(2):
                            i = 2 * g + b
                            mb = i - 4 * j
                            for t in range(max(0, mb), 4):
                                nc.tensor.matmul(
                                    ps_avq[:, t, :],
                                    lhsT=exp_t[:, b, 128 * t:128 * (t + 1)],
                                    rhs=v_aug[:, i, h, :],
                                    start=False,
                                    stop=(i == 4 * j + t),
                                    skip_group_check=True,
                                )
                            if tail and mb >= 0:
                                # chain mb just stopped: normalize on DVE now
                                norm_subtile(mb)
                            if tail and mb >= 1:
                                # chain mb-1 normalized one b-step ago ->
                                # transpose + close its projection m-tile
                                tail_subtile(mb - 1, psT)
                    if tail:
                        # drain remaining fillers (their y DMAs must precede
                        # the final m-tile's), then close the last subtile
                        reserve[0] = 0
                        while fillers:
                            fillers.pop(0)[1]()
                        tail_subtile(3, psT)
                    elif h % 2 == 0:
                        for t in range(4):
                            norm_subtile(t)
                    else:
                        psT = p_pst.tile([128, 4, 128], BF16, tag="pst")
                        for t in range(4):
                            norm_subtile(t)
                        # fillers between the DVE normalizes and the PE
                        # transposes hide the normalize latency
                        pump(2)
                        for t in range(4):
                            nc.tensor.transpose(
                                psT[:, t, :], asb[:, t, :], ident_sb[:, :]
                            )
                        nc.vector.tensor_copy(attn2[:, pi, :], psT[:, :, :])
                    # drain PE filler work into the ACT-paced stretch,
                    # counting only PE (matmul) steps toward the quota
                    if h >= 1 and not tail:
                        npe = sum(1 for is_pe, _ in fillers if is_pe)
                        take = max(1, (npe - reserve[0]) // (6 * (NH - h)))
                        while fillers and take > 0:
                            if reserve[0] and sum(
                                1 for p, _ in fillers if p
                            ) <= reserve[0]:
                                break
                            is_pe, fn = fillers.pop(0)
                            fn()
                            if is_pe:
                                take -= 1
                while fillers:
                    fillers.pop(0)[1]()

            def proj_steps_m(j, m):
                """Micro-steps for one 128-row tile of the out-projection.
                Each 512-col chain (both pair accumulations) is one atomic
                step so a pump boundary never leaves a PSUM chain open while
                other code allocates from the same pool."""
                o = (m % 4) * 128
                cell = {}

                def mmv(n):
                    if n == 0:
                        cell["y"] = p_y.tile([128, 2, 512], BF16, tag="y",
                                             name="y_sb")
                    ps = p_misc.tile([128, 512], F32, tag="m", name="ps_y")
                    for pp in range(NPAIR):
                        nc.tensor.matmul(
                            ps[:, :],
                            lhsT=attn2[:, 2 * j + pp, o:o + 128],
                            rhs=wproj_sb[:, pp, n * 512:(n + 1) * 512],
                            start=(pp == 0),
                            stop=(pp == NPAIR - 1),
                        )
                    nc.vector.tensor_copy(cell["y"][:, n, :], ps[:, :])

                def out():
                    nc.sync.dma_start(
                        out=y_d[m * 128:(m + 1) * 128, :],
                        in_=cell["y"][:, :, :],
                    )

                steps = []
                for n in range(2):
                    steps.append((True, lambda n=n: mmv(n)))
                steps.append((False, out))
                return steps

            def proj_steps(j):
                steps = []
                for m in range(j * 4, j * 4 + 4):
                    steps += proj_steps_m(j, m)
                return steps

            # j=0 prologue. The startup is DMA-serial-bound: run all four qk
            # tiles chunk-half-major (4 concurrent PSUM chains, borrowing the
            # idle score pool) so PE tracks the half-chunk DMA cadence; then
            # v chunk-major the same way.
            junk(75)
            ps_qk0 = p_misc.tile([128, 512], F32, tag="m")
            ps_qk1 = p_misc.tile([128, 512], F32, tag="m")
            ps_qk23 = p_s.tile([128, 2, 512], F32, tag="s")
            chains = (ps_qk0[:, :], ps_qk1[:, :], ps_qk23[:, 0, :],
                      ps_qk23[:, 1, :])
            for kh in range(2):
                for k in range(kh * KH, (kh + 1) * KH):
                    for mp in range(NM):
                        nc.tensor.matmul(
                            chains[mp],
                            lhsT=wqk_sb[:, k, mp * 128:(mp + 1) * 128],
                            rhs=xs0[:, k, :],
                            start=(k == 0),
                            stop=(k == KD - 1),
                        )
            for mp in range(NM):
                qk_move(0, mp, chains[mp])
            ps_v01 = p_s.tile([128, 2, 512], F32, tag="s")
            vchains = (ps_v01[:, 0, 0:256], ps_v01[:, 1, 0:256], None, None)
            vcells = [None, None, None, None]
            for k in range(KD):
                for m in range(4):
                    if m < 2:
                        ps = vchains[m]
                    else:
                        if k == 0 and vcells[m] is None:
                            vcells[m] = p_misc.tile(
                                [128, NH * 64], F32, tag="m", name="ps_v"
                            )
                        ps = vcells[m][:, :]
                    nc.tensor.matmul(
                        ps,
                        lhsT=xs0[:, k, m * 128:m * 128 + 128],
                        rhs=wv_sb[:, k, :],
                        start=(k == 0),
                        stop=(k == KD - 1),
                    )
            for m in range(4):
                src = vchains[m] if m < 2 else vcells[m][:, :]
                nc.vector.tensor_add(
                    v_aug[:, m, :, 0:64],
                    src.rearrange("p (h c) -> p h c", c=64),
                    bvbc_sb.rearrange("p (h c) -> p h c", c=64),
                )
            xs_next = xs1 if SQB > 1 else None
            for j in range(SQB):
                fillers = []
                if j + 1 < SQB:
                    fillers += qkv_steps(j + 1, xs_next)
                    xs_after = load_xs(j + 2) if j + 2 < SQB else None
                else:
                    xs_after = None
                if j == SQB - 1:
                    for jp in range(SQB - 1):
                        fillers += proj_steps(jp)
                attention_block(j, fillers)
                xs_next = xs_after

    return nc


def make_mask4():
    p = np.arange(128)[:, None]
    f = np.arange(128)[None, :]
    return (f >= p).astype(np.float32).copy()  # [128, 128] lower-tri in T layout


def to_bf16(x):
    import ml_dtypes

    return np.asarray(x, dtype=np.float32).astype(ml_dtypes.bfloat16)


def make_in_maps(x, W_qkv, b_qkv, W_proj):
    """Per-core input dicts for the full-size problem (bf16 staged)."""
    mask4 = make_mask4()
    ident = np.eye(128, dtype=np.float32)
    in_maps = []
    for c in range(NCORES):
        b, q = c // 4, c % 4
        cq = slice(256 * q, 256 * q + 256)
        wqk = np.concatenate([W_qkv[:, cq], W_qkv[:, 1024:2048][:, cq]], axis=1)
        wv = W_qkv[:, 2048:3072][:, cq]
        bqk = np.concatenate([b_qkv[cq], b_qkv[1024:2048][cq]]).reshape(4, 128)
        bvbc = np.broadcast_to(b_qkv[2048:3072][cq], (128, 256))
        # packed misc input: [128, 4 bqk-cols | 256 bvbc | 128 mask | 128 id]
        misc = np.concatenate([bqk.T, bvbc, mask4, ident], axis=1).astype(
            np.float32
        )
        # pair-packed wproj: [p, pp, n] = W_proj[256*q + pp*128 + p, n]
        wproj = np.ascontiguousarray(
            W_proj[cq, :].reshape(2, 128, 1024).transpose(1, 0, 2)
        )
        in_maps.append(
            {
                "xT": np.ascontiguousarray(to_bf16(x[b].T)),
                "wqk": np.ascontiguousarray(to_bf16(wqk)),
                "wv": np.ascontiguousarray(to_bf16(wv)),
                "misc": np.ascontiguousarray(misc),
                "wproj": to_bf16(wproj),
            }
        )
    return in_maps


_NC_CACHE = {}


def _get_nc():
    if "nc" not in _NC_CACHE:
        _NC_CACHE["nc"] = build_nc()
    return _NC_CACHE["nc"]


def run_on_hw(x, W_qkv, b_qkv, W_proj, b_proj, trace=False, **trace_kw):
    from concourse.bass_utils import run_bass_kernel_spmd

    in_maps = make_in_maps(x, W_qkv, b_qkv, W_proj)
    res = run_bass_kernel_spmd(
        _get_nc(), in_maps, core_ids=list(range(NCORES)), trace=trace, **trace_kw
    )
    out = np.empty((B, S, D), dtype=np.float32)
    for b in range(B):
        acc = res.results[4 * b]["y"].astype(np.float32)
        for q in range(1, 4):
            acc = acc + res.results[4 * b + q]["y"].astype(np.float32)
        out[b] = acc + b_proj[None, :]
    return out, res


def kernel(x, W_qkv, b_qkv, W_proj, b_proj):
    x = np.asarray(x, dtype=np.float32)
    W_qkv = np.asarray(W_qkv, dtype=np.float32)
    b_qkv = np.asarray(b_qkv, dtype=np.float32)
    W_proj = np.asarray(W_proj, dtype=np.float32)
    b_proj = np.asarray(b_proj, dtype=np.float32)
    out, _ = run_on_hw(x, W_qkv, b_qkv, W_proj, b_proj, trace=False)
    return out


# revision 31
# speedup vs baseline: 1.0923x; 1.0007x over previous
"""Trainium2 Bass kernel v3 for causal multi-head self-attention.

Problem (hardcoded):
    x:      [2, 2048, 1024] f32
    W_qkv:  [1024, 3072] f32   (cols: [q | k | v], each 1024 = 16 heads x 64)
    b_qkv:  [3072] f32
    W_proj: [1024, 1024] f32
    b_proj: [1024] f32
    out:    [2, 2048, 1024] f32

Sharding over 8 NeuronCores: data parallel on batch (2) x tensor parallel on
heads (4 quads of 4 heads). Core c handles batch c//4, heads [4*(c%4), 4*(c%4)+4).
Host gather sums the 4 partial projections per batch and adds b_proj.

v3 changes vs v2:
  - AV matmul orientation flipped: out[q,hd] accumulators with exp weights as
    the stationary operand and the 65-col V(+ones) as the moving operand.
    Halves AV's PE column charge (the cost model charges moving columns only).
  - softmax normalize becomes a per-partition-scalar multiply (reciprocal of
    the ones-column sums + tensor_scalar_mul on the PSUM->SBUF copy); the
    DRAM-bounce broadcast, gpsimd multiplies, and scratch tensor are gone.
  - a cheap PE transpose (identity matmul, 128 cols/tile) restores the
    [hd, q] layout the out-projection consumes; pair-packed attn2 unchanged.
  - last block's tail is pipelined per 128-query subtile: each AV chain stops
    early, normalizes, transposes, and feeds its projection m-tile while the
    remaining chains still accumulate.
"""

import os
import sys

for _p in ("/opt/trn_rl_repo", "/root/.axon_site/_ro/trn_rl_repo"):
    if os.path.isdir(_p) and _p not in sys.path:
        sys.path.append(_p)

import numpy as np

import concourse.bass as bass
import concourse.mybir as mybir
import concourse.tile as tile
from concourse.alu_op_type import AluOpType

F32 = mybir.dt.float32
BF16 = mybir.dt.bfloat16
AFT = mybir.ActivationFunctionType

B, S, D, H, HD = 2, 2048, 1024, 16, 64
NCORES = 8
NH = 4  # heads per core
SCALE = 1.0 / 8.0  # 1/sqrt(64)


class SplitWaitTileContext(tile.TileContext):
    """This container's walrus rejects >1 sync wait per instruction
    ("Too many sync wait commands"). Split extra waits onto preceding
    same-engine NoOps before the final block lowering."""

    def _lower_ordered_insts(self, ordered):
        for bb_name, insts in list(ordered.items()):
            new = []
            for inst in insts:
                si = inst.sync_info
                if si is not None and si.on_wait and len(si.on_wait) > 1:
                    waits = list(si.on_wait)
                    for w in waits[:-1]:
                        nop = mybir.InstNoOp(
                            name=f"nopw-{self.nc.get_next_instruction_name()}"
                        )
                        nop.engine = inst.engine
                        nop.sync_info = mybir.SyncInfo(on_wait=[w], on_update=[])
                        new.append(nop)
                    inst.sync_info = mybir.SyncInfo(
                        on_wait=[waits[-1]], on_update=list(si.on_update or [])
                    )
                new.append(inst)
            ordered[bb_name] = new
        return super()._lower_ordered_insts(ordered)

    def _drain_and_barrier(self, tick_clock, wait_clock):
        from concourse.vector_clock import ScopedClock

        drain_inst = self.nc.sync.drain()
        wait_clock.add_sem_waits(
            drain_inst.ins, ScopedClock({None: tick_clock.global_clock})
        )
        si = drain_inst.ins.sync_info
        if si is not None and si.on_wait and len(si.on_wait) > 1:
            waits = list(si.on_wait)
            drain_inst.ins.sync_info = mybir.SyncInfo(
                on_wait=[waits[0]], on_update=list(si.on_update or [])
            )
            for w in waits[1:]:
                nop = self.nc.sync.nop(nofuse=True)
                nop.ins.sync_info = mybir.SyncInfo(on_wait=[w], on_update=[])

        self.nc.all_engine_barrier()
        assert self.sems is not None
        popped = self.nc._tile_sem_poison_stack.pop()
        assert popped is self._sem_poison
        self.nc.clear_and_free_semaphores(list(self.sems.allocated().values()))
        self.nc.all_engine_barrier()


def build_nc(S=S, D=D, NH=NH, dbg=False, reps=1):
    """Build the single-core SPMD program."""
    KD = D // 128             # 8 k-chunks of the D contraction
    NM = NH * 2 * 64 // 128   # 4 qk M-tiles (q chunks then k chunks)
    NMQ = NM // 2
    SQB = S // 512            # 4 sq blocks of 512
    NSK = S // 128            # 16 sk tiles of 128
    NPAIR = NH // 2           # 2 head pairs per block
    NPJ = SQB * NPAIR         # 8 pair-slots (pair index pi = 2*j + pp)
    MISC_W = NM + NH * 64 + 128 + 128  # bqk | bvbc | mask | identity

    nc = bass.Bass("TRN2", target_bir_lowering=False, debug=False)

    xT_d = nc.dram_tensor("xT", [D, S], BF16, kind="ExternalInput").ap()
    wqk_d = nc.dram_tensor("wqk", [D, NM * 128], BF16, kind="ExternalInput").ap()
    wv_d = nc.dram_tensor("wv", [D, NH * 64], BF16, kind="ExternalInput").ap()
    misc_d = nc.dram_tensor("misc", [128, MISC_W], F32, kind="ExternalInput").ap()
    # wproj pre-packed on host: [p, pp, n] = W_proj[core_base + pp*128 + p, n]
    wproj_d = nc.dram_tensor("wproj", [128, NPAIR, D], BF16, kind="ExternalInput").ap()
    y_d = nc.dram_tensor("y", [S, D], BF16, kind="ExternalOutput").ap()

    with SplitWaitTileContext(nc) as tc:
        with (
            nc.allow_low_precision(reason="bf16 feeds PE; fp32 accum in PSUM"),
            tc.tile_pool(name="stream", bufs=2) as p_stream,
            tc.tile_pool(name="attnp", bufs=1) as p_attn,
            tc.tile_pool(name="wpool", bufs=1) as p_w,
            tc.tile_pool(name="qkt", bufs=1) as p_qkt,
            tc.tile_pool(name="vaug", bufs=1) as p_vaug,
            tc.tile_pool(name="expp", bufs=8) as p_exp,
            tc.tile_pool(name="asb", bufs=2) as p_asb,
            tc.tile_pool(name="rcpp", bufs=2) as p_rcp,
            tc.tile_pool(name="ypool", bufs=8) as p_y,
            tc.tile_pool(name="pmisc", bufs=2, space="PSUM") as p_misc,
            tc.tile_pool(name="ps", bufs=2, space="PSUM") as p_s,
            tc.tile_pool(name="pavq", bufs=1, space="PSUM") as p_avq,
            tc.tile_pool(name="ppst", bufs=1, space="PSUM") as p_pst,
        ):
          for _rep in range(reps):
            # PE warmup: junk matmuls keep the systolic array ramped while the
            # input DMAs land
            ones_sb = p_w.tile([128, 260], BF16, tag="ones")
            # junk only needs the first 64 cols; memset those first so the
            # PE warmup starts as early as possible
            nc.vector.memset(ones_sb[:, 0:64], 1.0)
            nc.vector.memset(ones_sb[:, 64:260], 1.0)
            zero_sb = p_w.tile([128, 128], BF16, tag="zero")
            nc.vector.memset(zero_sb[:, :], 0.0)
            # preload the exp table set in the startup window
            expwarm = p_w.tile([1, 1], F32, tag="expwarm")
            nc.scalar.activation(
                expwarm[:, :], ones_sb[0:1, 0:1], AFT.Exp, scale=SCALE
            )
            warm_ps = p_avq.tile([128, 4, 65], F32, tag="avq", name="warm_ps")

            def junk(n):
                # junk may leave stale pending-zero flags in the bank; the
                # per-head zero matmul re-marks and clears every chain byte,
                # so junk's footprint doesn't matter
                for _ in range(n):
                    nc.tensor.matmul(
                        warm_ps[0:64, 0, 0:64],
                        lhsT=ones_sb[:, 0:64],
                        rhs=ones_sb[:, 0:64],
                        start=True,
                        stop=True,
                    )

            junk(40)

            # input DMAs: transfers AND descriptor-gen (627ns HWDGE) serialize
            # device-wide, so the first-needed big transfers go first; tiny
            # bias/mask loads ride behind them (needed only ~10us in)
            xT_src = xT_d.rearrange("(c p) s -> p c s", p=128)
            xs0 = p_stream.tile([128, KD, 512], BF16, tag="xs")
            wqk_sb = p_w.tile([128, KD, NM * 128], BF16, tag="wqk")
            wqk_src = wqk_d.rearrange("(c p) n -> p c n", p=128)
            # 4-chunk granularity balances serialized HWDGE descriptor-gen
            # against time-to-first-chunk
            KH = KD // 2
            nc.sync.dma_start(out=xs0[:, 0:KH, :], in_=xT_src[:, 0:KH, 0:512])
            nc.sync.dma_start(out=wqk_sb[:, 0:KH, :], in_=wqk_src[:, 0:KH, :])
            nc.sync.dma_start(out=xs0[:, KH:KD, :], in_=xT_src[:, KH:KD, 0:512])
            nc.sync.dma_start(out=wqk_sb[:, KH:KD, :], in_=wqk_src[:, KH:KD, :])

            misc_sb = p_w.tile([128, MISC_W], F32, tag="misc")
            nc.sync.dma_start(out=misc_sb[:, :], in_=misc_d[:, :])
            bqk_sb = misc_sb[:, 0:NM]
            bvbc_sb = misc_sb[:, NM:NM + NH * 64]
            mask_sb = p_w.tile([128, 128], BF16, tag="mask")
            nc.vector.tensor_copy(
                mask_sb[:, :], misc_sb[:, NM + NH * 64:NM + NH * 64 + 128]
            )
            ident_sb = p_w.tile([128, 128], BF16, tag="ident")
            nc.vector.tensor_copy(
                ident_sb[:, :], misc_sb[:, NM + NH * 64 + 128:MISC_W]
            )

            wv_sb = p_w.tile([128, KD, NH * 64], BF16, tag="wv")
            wv_src = wv_d.rearrange("(c p) n -> p c n", p=128)
            nc.sync.dma_start(out=wv_sb[:, :, :], in_=wv_src[:, :, :])

            xs1 = p_stream.tile([128, KD, 512], BF16, tag="xs", name="xs1")
            nc.sync.dma_start(out=xs1[:, :, :], in_=xT_src[:, :, 512:1024])

            # wproj is only needed by the projection fillers in the final
            # block (~90us in); load it behind everything the front needs
            wproj_sb = p_w.tile([128, NPAIR, D], BF16, tag="wproj")
            nc.sync.dma_start(out=wproj_sb[:, :, :], in_=wproj_d[:, :, :])

            qkT_sb = p_qkt.tile([128, NM, S], BF16, tag="qkt")
            v_aug = p_vaug.tile([128, NSK, NH, 65], BF16, tag="vaug")
            nc.vector.memset(v_aug[:, :, :, 64:65], 1.0)
            # attn2: pair-packed normalized attnT. partitions 0:64 head 2pp,
            # 64:128 head 2pp+1; slot pi = 2*j + pp
            attn2 = p_attn.tile([128, NPJ, 512], BF16, tag="attn")

            def load_xs(j):
                xs = p_stream.tile([128, KD, 512], BF16, tag="xs")
                nc.sync.dma_start(
                    out=xs[:, :, :], in_=xT_src[:, :, j * 512:(j + 1) * 512]
                )
                return xs

            def qk_move(j, mp, ps_qk):
                dst = qkT_sb[:, mp, j * 512:(j + 1) * 512]
                nc.vector.tensor_scalar_add(dst, ps_qk[:, :], bqk_sb[:, mp:mp + 1])

            def qk_steps(j, xs, mp):
                """Micro-steps (one matmul each) for one qk projection tile."""
                cell = {}

                def mm(k):
                    if k == 0:
                        cell["ps"] = p_misc.tile([128, 512], F32, tag="m", name="ps_qk")
                    nc.tensor.matmul(
                        cell["ps"][:, :],
                        lhsT=wqk_sb[:, k, mp * 128:(mp + 1) * 128],
                        rhs=xs[:, k, :],
                        start=(k == 0),
                        stop=(k == KD - 1),
                    )

                return [(True, lambda k=k: mm(k)) for k in range(KD)] + [
                    (False, lambda: qk_move(j, mp, cell["ps"]))
                ]

            def v_steps(j, xs, m):
                cell = {}

                def mm(k):
                    if k == 0:
                        cell["ps"] = p_misc.tile([128, NH * 64], F32, tag="m", name="ps_v")
                    nc.tensor.matmul(
                        cell["ps"][:, :],
                        lhsT=xs[:, k, (m % 4) * 128:(m % 4) * 128 + 128],
                        rhs=wv_sb[:, k, :],
                        start=(k == 0),
                        stop=(k == KD - 1),
                    )

                def mv():
                    nc.vector.tensor_add(
                        v_aug[:, m, :, 0:64],
                        cell["ps"][:, :].rearrange("p (h c) -> p h c", c=64),
                        bvbc_sb.rearrange("p (h c) -> p h c", c=64),
                    )

                return [(True, lambda k=k: mm(k)) for k in range(KD)] + [
                    (False, mv)
                ]

            def qkv_steps(j, xs):
                steps = []
                for mp in range(NM):
                    steps += qk_steps(j, xs, mp)
                for m in range(4 * j, 4 * j + 4):
                    steps += v_steps(j, xs, m)
                return steps

            def attention_block(j, fillers):
                # the final block's ACT chain is saturated; PE idle there is
                # free, so spend fillers early and keep the tail clean.
                # Early blocks have few groups but many filler steps: pump
                # hard so the next block's qk tiles (and their DVE moves)
                # finish long before the block boundary.
                reserve = [0]
                pump_n = {0: 6, 1: 3, 2: 2}.get(j, 2)

                def pump(n=1):
                    got = 0
                    while fillers and got < n:
                        if reserve[0] and sum(
                            1 for p, _ in fillers if p
                        ) <= reserve[0]:
                            return
                        is_pe, fn = fillers.pop(0)
                        fn()
                        if is_pe:
                            got += 1

                def emit_scores_h(hh, g):
                    # exact causal regions; the merged diag exp also reads
                    # stale PSUM outside them, which downstream AV never
                    # consumes (harmless garbage, skipped per subtile)
                    qTh = qkT_sb[64 * (hh % 2):64 * (hh % 2) + 64, hh // 2, :]
                    kTh = qkT_sb[64 * (hh % 2):64 * (hh % 2) + 64,
                                 NMQ + hh // 2, :]
                    ps = p_s.tile([128, 2, 512], F32, tag="s")
                    for b in range(2):
                        i = 2 * g + b
                        mb = i - 4 * j
                        no = 0 if mb <= 0 else 128 * mb
                        nc.tensor.matmul(
                            ps[:, b, no:512],
                            lhsT=kTh[:, i * 128:(i + 1) * 128],
                            rhs=qTh[:, j * 512 + no:(j + 1) * 512],
                            start=True,
                            stop=True,
                        )
                    return ps

                carry = None
                for h in range(NH):
                    ps_avq = p_avq.tile([128, 4, 65], F32, tag="avq")
                    # the 4 AV chains share one PSUM bank; a per-chain matmul
                    # start=True would mark the whole 2KB bank pending-zero
                    # and wipe its neighbours' partials. Open the bank with
                    # one zero matmul covering every chain byte, then
                    # accumulate with start=False throughout.
                    nc.tensor.matmul(
                        ps_avq[:, :, :],
                        lhsT=zero_sb[:, :],
                        rhs=ones_sb[:, :],
                        start=True,
                        stop=False,
                        skip_group_check=True,
                    )
                    pp = h // 2
                    pi = 2 * j + pp
                    if h % 2 == 0:
                        asb_cur = p_asb.tile([128, 4, 128], BF16, tag="asb")
                    asb = asb_cur
                    npair = 2 * (j + 1)
                    tail = j == SQB - 1 and h == NH - 1

                    rcp = p_rcp.tile([128, 4, 1], F32, tag="rcp")

                    def norm_subtile(t):
                        """reciprocal of the ones-column sum + normalized
                        PSUM->SBUF copy for q-subtile t of this head."""
                        nc.vector.reciprocal(
                            rcp[:, t:t + 1, :], ps_avq[:, t:t + 1, 64:65]
                        )
                        nc.vector.tensor_scalar_mul(
                            asb[:, t, 64 * (h % 2):64 * (h % 2) + 64],
                            ps_avq[:, t, 0:64],
                            rcp[:, t, :],
                        )

                    def tail_subtile(t, psT):
                        """Last head: chain t stopped one b-step ago and its
                        normalize already ran on DVE. Transpose and close
                        projection m-tile 12+t (both 512-col halves). ACT is
                        free after the final exp, so it takes the attn2 chunk
                        and one y half; DVE keeps the norms and the other y
                        half."""
                        nc.tensor.transpose(
                            psT[:, t, :], asb[:, t, :], ident_sb[:, :]
                        )
                        nc.scalar.copy(
                            attn2[:, pi, 128 * t:128 * (t + 1)], psT[:, t, :]
                        )
                        y_sb = p_y.tile([128, 2, 512], BF16, tag="y",
                                        name="y_sb")
                        chs = []
                        for n in range(2):
                            ch = p_misc.tile([128, 512], F32, tag="m",
                                             name="tp_ps")
                            chs.append(ch)
                            nc.tensor.matmul(
                                ch[:, :],
                                lhsT=attn2[:, 2 * j, t * 128:(t + 1) * 128],
                                rhs=wproj_sb[:, 0, n * 512:(n + 1) * 512],
                                start=True,
                                stop=False,
                            )
                        for n in range(2):
                            nc.tensor.matmul(
                                chs[n][:, :],
                                lhsT=attn2[:, 2 * j + 1,
                                           t * 128:(t + 1) * 128],
                                rhs=wproj_sb[:, 1, n * 512:(n + 1) * 512],
                                start=False,
                                stop=True,
                            )
                        m = 4 * j + t
                        nc.scalar.copy(y_sb[:, 0, :], chs[0][:, :])
                        nc.vector.tensor_copy(y_sb[:, 1, :], chs[1][:, :])
                        nc.sync.dma_start(
                            out=y_d[m * 128:(m + 1) * 128, :],
                            in_=y_sb[:, :, :],
                        )

                    sc_next = carry if carry is not None else emit_scores_h(h, 0)
                    carry = None
                    if tail:
                        psT = p_pst.tile([128, 4, 128], BF16, tag="pst")
                    for g in range(npair):
                        ps_sc = sc_next
                        # 1-deep software pipeline: next group's scores are
                        # emitted before this group's AV so PE runs them
                        # while ACT computes this group's exp. The pipeline
                        # carries across heads (same block) so ACT never
                        # bubbles at a head boundary.
                        if g + 1 < npair:
                            sc_next = emit_scores_h(h, g + 1)
                        elif h + 1 < NH:
                            carry = emit_scores_h(h + 1, 0)
                        # ACT runs ~350ns/group longer than this loop's PE
                        # work; pump filler steps so PE never idles on exp
                        pump(pump_n)
                        exp_t = p_exp.tile([128, 2, 512], BF16, tag="exp")
                        if g == 2 * j:
                            # diag pair mb=0,1: single exp over both tiles
                            nc.scalar.activation(
                                exp_t[:, :, :], ps_sc[:, :, :], AFT.Exp, scale=SCALE
                            )
                            nc.vector.tensor_mul(
                                exp_t[:, 0, 0:128], exp_t[:, 0, 0:128], mask_sb[:, :]
                            )
                            nc.vector.tensor_mul(
                                exp_t[:, 1, 128:256], exp_t[:, 1, 128:256],
                                mask_sb[:, :],
                            )
                        elif g == 2 * j + 1:
                            # mb=2,3: exp the computed 256:512 of both tiles
                            nc.scalar.activation(
                                exp_t[:, :, 256:512],
                                ps_sc[:, :, 256:512],
                                AFT.Exp,
                                scale=SCALE,
                            )
                            nc.vector.tensor_mul(
                                exp_t[:, 0, 256:384], exp_t[:, 0, 256:384],
                                mask_sb[:, :],
                            )
                            nc.vector.tensor_mul(
                                exp_t[:, 1, 384:512], exp_t[:, 1, 384:512],
                                mask_sb[:, :],
                            )
                        else:
                            nc.scalar.activation(
                                exp_t[:, :, :], ps_sc[:, :, :], AFT.Exp, scale=SCALE
                            )
                        for b in range(2):
                            i = 2 * g + b
                            mb = i - 4 * j
                            for t in range(max(0, mb), 4):
                                nc.tensor.matmul(
                                    ps_avq[:, t, :],
                                    lhsT=exp_t[:, b, 128 * t:128 * (t + 1)],
                                    rhs=v_aug[:, i, h, :],
                                    start=False,
                                    stop=(i == 4 * j + t),
                                    skip_group_check=True,
                                )
                            if tail and mb >= 0:
                                # chain mb just stopped: normalize on DVE now
                                norm_subtile(mb)
                            if tail and mb >= 1:
                                # chain mb-1 normalized one b-step ago ->
                                # transpose + close its projection m-tile
                                tail_subtile(mb - 1, psT)
                    if tail:
                        # drain remaining fillers (their y DMAs must precede
                        # the final m-tile's), then close the last subtile
                        reserve[0] = 0
                        while fillers:
                            fillers.pop(0)[1]()
                        tail_subtile(3, psT)
                    elif h % 2 == 0:
                        for t in range(4):
                            norm_subtile(t)
                    else:
                        psT = p_pst.tile([128, 4, 128], BF16, tag="pst")
                        for t in range(4):
                            norm_subtile(t)
                        # fillers between the DVE normalizes and the PE
                        # transposes hide the normalize latency
                        pump(2)
                        for t in range(4):
                            nc.tensor.transpose(
                                psT[:, t, :], asb[:, t, :], ident_sb[:, :]
                            )
                        nc.vector.tensor_copy(attn2[:, pi, :], psT[:, :, :])
                    # drain PE filler work into the ACT-paced stretch,
                    # counting only PE (matmul) steps toward the quota
                    if h >= 1 and not tail:
                        npe = sum(1 for is_pe, _ in fillers if is_pe)
                        take = max(1, (npe - reserve[0]) // (6 * (NH - h)))
                        while fillers and take > 0:
                            if reserve[0] and sum(
                                1 for p, _ in fillers if p
                            ) <= reserve[0]:
                                break
                            is_pe, fn = fillers.pop(0)
                            fn()
                            if is_pe:
                                take -= 1
                while fillers:
                    fillers.pop(0)[1]()

            def proj_steps_m(j, m):
                """Micro-steps for one 128-row tile of the out-projection.
                Each 512-col chain (both pair accumulations) is one atomic
                step so a pump boundary never leaves a PSUM chain open while
                other code allocates from the same pool."""
                o = (m % 4) * 128
                cell = {}

                def mmv(n):
                    if n == 0:
                        cell["y"] = p_y.tile([128, 2, 512], BF16, tag="y",
                                             name="y_sb")
                    ps = p_misc.tile([128, 512], F32, tag="m", name="ps_y")
                    for pp in range(NPAIR):
                        nc.tensor.matmul(
                            ps[:, :],
                            lhsT=attn2[:, 2 * j + pp, o:o + 128],
                            rhs=wproj_sb[:, pp, n * 512:(n + 1) * 512],
                            start=(pp == 0),
                            stop=(pp == NPAIR - 1),
                        )
                    nc.vector.tensor_copy(cell["y"][:, n, :], ps[:, :])

                def out():
                    nc.sync.dma_start(
                        out=y_d[m * 128:(m + 1) * 128, :],
                        in_=cell["y"][:, :, :],
                    )

                steps = []
                for n in range(2):
                    steps.append((True, lambda n=n: mmv(n)))
                steps.append((False, out))
                return steps

            def proj_steps(j):
                steps = []
                for m in range(j * 4, j * 4 + 4):
                    steps += proj_steps_m(j, m)
                return steps

            # j=0 prologue. The startup is DMA-serial-bound: run all four qk
            # tiles chunk-half-major (4 concurrent PSUM chains, borrowing the
            # idle score pool) so PE tracks the half-chunk DMA cadence; then
            # v chunk-major the same way.
            junk(75)
            ps_qk0 = p_misc.tile([128, 512], F32, tag="m")
            ps_qk1 = p_misc.tile([128, 512], F32, tag="m")
            ps_qk23 = p_s.tile([128, 2, 512], F32, tag="s")
            chains = (ps_qk0[:, :], ps_qk1[:, :], ps_qk23[:, 0, :],
                      ps_qk23[:, 1, :])
            for kh in range(2):
                for k in range(kh * KH, (kh + 1) * KH):
                    for mp in range(NM):
                        nc.tensor.matmul(
                            chains[mp],
                            lhsT=wqk_sb[:, k, mp * 128:(mp + 1) * 128],
                            rhs=xs0[:, k, :],
                            start=(k == 0),
                            stop=(k == KD - 1),
                        )
            for mp in range(NM):
                qk_move(0, mp, chains[mp])
            ps_v01 = p_s.tile([128, 2, 512], F32, tag="s")
            vchains = (ps_v01[:, 0, 0:256], ps_v01[:, 1, 0:256], None, None)
            vcells = [None, None, None, None]
            for k in range(KD):
                for m in range(4):
                    if m < 2:
                        ps = vchains[m]
                    else:
                        if k == 0 and vcells[m] is None:
                            vcells[m] = p_misc.tile(
                                [128, NH * 64], F32, tag="m", name="ps_v"
                            )
                        ps = vcells[m][:, :]
                    nc.tensor.matmul(
                        ps,
                        lhsT=xs0[:, k, m * 128:m * 128 + 128],
                        rhs=wv_sb[:, k, :],
                        start=(k == 0),
                        stop=(k == KD - 1),
                    )
            for m in range(4):
                src = vchains[m] if m < 2 else vcells[m][:, :]
                nc.vector.tensor_add(
                    v_aug[:, m, :, 0:64],
                    src.rearrange("p (h c) -> p h c", c=64),
                    bvbc_sb.rearrange("p (h c) -> p h c", c=64),
                )
            xs_next = xs1 if SQB > 1 else None
            for j in range(SQB):
                fillers = []
                if j + 1 < SQB:
                    fillers += qkv_steps(j + 1, xs_next)
                    xs_after = load_xs(j + 2) if j + 2 < SQB else None
                else:
                    xs_after = None
                if j == SQB - 1:
                    for jp in range(SQB - 1):
                        fillers += proj_steps(jp)
                attention_block(j, fillers)
                xs_next = xs_after

    return nc


def make_mask4():
    p = np.arange(128)[:, None]
    f = np.arange(128)[None, :]
    return (f >= p).astype(np.float32).copy()  # [128, 128] lower-tri in T layout


def to_bf16(x):
    import ml_dtypes

    return np.asarray(x, dtype=np.float32).astype(ml_dtypes.bfloat16)


def make_in_maps(x, W_qkv, b_qkv, W_proj):
    """Per-core input dicts for the full-size problem (bf16 staged)."""
    mask4 = make_mask4()
    ident = np.eye(128, dtype=np.float32)
    in_maps = []
    for c in range(NCORES):
        b, q = c // 4, c % 4
        cq = slice(256 * q, 256 * q + 256)
        wqk = np.concatenate([W_qkv[:, cq], W_qkv[:, 1024:2048][:, cq]], axis=1)
        wv = W_qkv[:, 2048:3072][:, cq]
        bqk = np.concatenate([b_qkv[cq], b_qkv[1024:2048][cq]]).reshape(4, 128)
        bvbc = np.broadcast_to(b_qkv[2048:3072][cq], (128, 256))
        # packed misc input: [128, 4 bqk-cols | 256 bvbc | 128 mask | 128 id]
        misc = np.concatenate([bqk.T, bvbc, mask4, ident], axis=1).astype(
            np.float32
        )
        # pair-packed wproj: [p, pp, n] = W_proj[256*q + pp*128 + p, n]
        wproj = np.ascontiguousarray(
            W_proj[cq, :].reshape(2, 128, 1024).transpose(1, 0, 2)
        )
        in_maps.append(
            {
                "xT": np.ascontiguousarray(to_bf16(x[b].T)),
                "wqk": np.ascontiguousarray(to_bf16(wqk)),
                "wv": np.ascontiguousarray(to_bf16(wv)),
                "misc": np.ascontiguousarray(misc),
                "wproj": to_bf16(wproj),
            }
        )
    return in_maps


_NC_CACHE = {}


def _get_nc():
    if "nc" not in _NC_CACHE:
        _NC_CACHE["nc"] = build_nc()
    return _NC_CACHE["nc"]


def run_on_hw(x, W_qkv, b_qkv, W_proj, b_proj, trace=False, **trace_kw):
    from concourse.bass_utils import run_bass_kernel_spmd

    in_maps = make_in_maps(x, W_qkv, b_qkv, W_proj)
    res = run_bass_kernel_spmd(
        _get_nc(), in_maps, core_ids=list(range(NCORES)), trace=trace, **trace_kw
    )
    out = np.empty((B, S, D), dtype=np.float32)
    for b in range(B):
        acc = res.results[4 * b]["y"].astype(np.float32)
        for q in range(1, 4):
            acc = acc + res.results[4 * b + q]["y"].astype(np.float32)
        out[b] = acc + b_proj[None, :]
    return out, res


def kernel(x, W_qkv, b_qkv, W_proj, b_proj):
    x = np.asarray(x, dtype=np.float32)
    W_qkv = np.asarray(W_qkv, dtype=np.float32)
    b_qkv = np.asarray(b_qkv, dtype=np.float32)
    W_proj = np.asarray(W_proj, dtype=np.float32)
    b_proj = np.asarray(b_proj, dtype=np.float32)
    out, _ = run_on_hw(x, W_qkv, b_qkv, W_proj, b_proj, trace=False)
    return out


# revision 33
# speedup vs baseline: 1.0931x; 1.0007x over previous
"""Trainium2 Bass kernel v3 for causal multi-head self-attention.

Problem (hardcoded):
    x:      [2, 2048, 1024] f32
    W_qkv:  [1024, 3072] f32   (cols: [q | k | v], each 1024 = 16 heads x 64)
    b_qkv:  [3072] f32
    W_proj: [1024, 1024] f32
    b_proj: [1024] f32
    out:    [2, 2048, 1024] f32

Sharding over 8 NeuronCores: data parallel on batch (2) x tensor parallel on
heads (4 quads of 4 heads). Core c handles batch c//4, heads [4*(c%4), 4*(c%4)+4).
Host gather sums the 4 partial projections per batch and adds b_proj.

v3 changes vs v2:
  - AV matmul orientation flipped: out[q,hd] accumulators with exp weights as
    the stationary operand and the 65-col V(+ones) as the moving operand.
    Halves AV's PE column charge (the cost model charges moving columns only).
  - softmax normalize becomes a per-partition-scalar multiply (reciprocal of
    the ones-column sums + tensor_scalar_mul on the PSUM->SBUF copy); the
    DRAM-bounce broadcast, gpsimd multiplies, and scratch tensor are gone.
  - a cheap PE transpose (identity matmul, 128 cols/tile) restores the
    [hd, q] layout the out-projection consumes; pair-packed attn2 unchanged.
  - last block's tail is pipelined per 128-query subtile: each AV chain stops
    early, normalizes, transposes, and feeds its projection m-tile while the
    remaining chains still accumulate.
"""

import os
import sys

for _p in ("/opt/trn_rl_repo", "/root/.axon_site/_ro/trn_rl_repo"):
    if os.path.isdir(_p) and _p not in sys.path:
        sys.path.append(_p)

import numpy as np

import concourse.bass as bass
import concourse.mybir as mybir
import concourse.tile as tile
from concourse.alu_op_type import AluOpType

F32 = mybir.dt.float32
BF16 = mybir.dt.bfloat16
AFT = mybir.ActivationFunctionType

B, S, D, H, HD = 2, 2048, 1024, 16, 64
NCORES = 8
NH = 4  # heads per core
SCALE = 1.0 / 8.0  # 1/sqrt(64)


class SplitWaitTileContext(tile.TileContext):
    """This container's walrus rejects >1 sync wait per instruction
    ("Too many sync wait commands"). Split extra waits onto preceding
    same-engine NoOps before the final block lowering."""

    def _lower_ordered_insts(self, ordered):
        for bb_name, insts in list(ordered.items()):
            new = []
            for inst in insts:
                si = inst.sync_info
                if si is not None and si.on_wait and len(si.on_wait) > 1:
                    waits = list(si.on_wait)
                    for w in waits[:-1]:
                        nop = mybir.InstNoOp(
                            name=f"nopw-{self.nc.get_next_instruction_name()}"
                        )
                        nop.engine = inst.engine
                        nop.sync_info = mybir.SyncInfo(on_wait=[w], on_update=[])
                        new.append(nop)
                    inst.sync_info = mybir.SyncInfo(
                        on_wait=[waits[-1]], on_update=list(si.on_update or [])
                    )
                new.append(inst)
            ordered[bb_name] = new
        return super()._lower_ordered_insts(ordered)

    def _drain_and_barrier(self, tick_clock, wait_clock):
        from concourse.vector_clock import ScopedClock

        drain_inst = self.nc.sync.drain()
        wait_clock.add_sem_waits(
            drain_inst.ins, ScopedClock({None: tick_clock.global_clock})
        )
        si = drain_inst.ins.sync_info
        if si is not None and si.on_wait and len(si.on_wait) > 1:
            waits = list(si.on_wait)
            drain_inst.ins.sync_info = mybir.SyncInfo(
                on_wait=[waits[0]], on_update=list(si.on_update or [])
            )
            for w in waits[1:]:
                nop = self.nc.sync.nop(nofuse=True)
                nop.ins.sync_info = mybir.SyncInfo(on_wait=[w], on_update=[])

        self.nc.all_engine_barrier()
        assert self.sems is not None
        popped = self.nc._tile_sem_poison_stack.pop()
        assert popped is self._sem_poison
        self.nc.clear_and_free_semaphores(list(self.sems.allocated().values()))
        self.nc.all_engine_barrier()


def build_nc(S=S, D=D, NH=NH, dbg=False, reps=1):
    """Build the single-core SPMD program."""
    KD = D // 128             # 8 k-chunks of the D contraction
    NM = NH * 2 * 64 // 128   # 4 qk M-tiles (q chunks then k chunks)
    NMQ = NM // 2
    SQB = S // 512            # 4 sq blocks of 512
    NSK = S // 128            # 16 sk tiles of 128
    NPAIR = NH // 2           # 2 head pairs per block
    NPJ = SQB * NPAIR         # 8 pair-slots (pair index pi = 2*j + pp)
    MISC_W = NM + NH * 64 + 128 + 128  # bqk | bvbc | mask | identity

    nc = bass.Bass("TRN2", target_bir_lowering=False, debug=False)

    xT_d = nc.dram_tensor("xT", [D, S], BF16, kind="ExternalInput").ap()
    wqk_d = nc.dram_tensor("wqk", [D, NM * 128], BF16, kind="ExternalInput").ap()
    wv_d = nc.dram_tensor("wv", [D, NH * 64], BF16, kind="ExternalInput").ap()
    misc_d = nc.dram_tensor("misc", [128, MISC_W], F32, kind="ExternalInput").ap()
    # wproj pre-packed on host: [p, pp, n] = W_proj[core_base + pp*128 + p, n]
    wproj_d = nc.dram_tensor("wproj", [128, NPAIR, D], BF16, kind="ExternalInput").ap()
    y_d = nc.dram_tensor("y", [S, D], BF16, kind="ExternalOutput").ap()

    with SplitWaitTileContext(nc) as tc:
        with (
            nc.allow_low_precision(reason="bf16 feeds PE; fp32 accum in PSUM"),
            tc.tile_pool(name="stream", bufs=2) as p_stream,
            tc.tile_pool(name="attnp", bufs=1) as p_attn,
            tc.tile_pool(name="wpool", bufs=1) as p_w,
            tc.tile_pool(name="qkt", bufs=1) as p_qkt,
            tc.tile_pool(name="vaug", bufs=1) as p_vaug,
            tc.tile_pool(name="expp", bufs=8) as p_exp,
            tc.tile_pool(name="asb", bufs=2) as p_asb,
            tc.tile_pool(name="rcpp", bufs=2) as p_rcp,
            tc.tile_pool(name="ypool", bufs=8) as p_y,
            tc.tile_pool(name="pmisc", bufs=2, space="PSUM") as p_misc,
            tc.tile_pool(name="ps", bufs=2, space="PSUM") as p_s,
            tc.tile_pool(name="pavq", bufs=1, space="PSUM") as p_avq,
            tc.tile_pool(name="ppst", bufs=1, space="PSUM") as p_pst,
        ):
          for _rep in range(reps):
            # PE warmup: junk matmuls keep the systolic array ramped while the
            # input DMAs land
            ones_sb = p_w.tile([128, 260], BF16, tag="ones")
            # junk only needs the first 64 cols; memset those first so the
            # PE warmup starts as early as possible
            nc.vector.memset(ones_sb[:, 0:64], 1.0)
            nc.vector.memset(ones_sb[:, 64:260], 1.0)
            zero_sb = p_w.tile([128, 128], BF16, tag="zero")
            nc.vector.memset(zero_sb[:, :], 0.0)
            # preload the exp table set in the startup window
            expwarm = p_w.tile([1, 1], F32, tag="expwarm")
            nc.scalar.activation(
                expwarm[:, :], ones_sb[0:1, 0:1], AFT.Exp, scale=SCALE
            )
            warm_ps = p_avq.tile([128, 4, 65], F32, tag="avq", name="warm_ps")

            def junk(n):
                # junk may leave stale pending-zero flags in the bank; the
                # per-head zero matmul re-marks and clears every chain byte,
                # so junk's footprint doesn't matter
                for _ in range(n):
                    nc.tensor.matmul(
                        warm_ps[0:64, 0, 0:64],
                        lhsT=ones_sb[:, 0:64],
                        rhs=ones_sb[:, 0:64],
                        start=True,
                        stop=True,
                    )

            junk(40)

            # input DMAs: transfers AND descriptor-gen (627ns HWDGE) serialize
            # device-wide. wqk is packed [q01|k01|q23|k23] on the host so the
            # first half (cols 0:256) is exactly what heads 0,1 of block 0
            # need - attention can start before the rest of wqk lands.
            xT_src = xT_d.rearrange("(c p) s -> p c s", p=128)
            xs0 = p_stream.tile([128, KD, 512], BF16, tag="xs")
            wqk_sb = p_w.tile([128, KD, NM * 128], BF16, tag="wqk")
            wqk_src = wqk_d.rearrange("(c p) n -> p c n", p=128)
            KH = KD // 2
            misc_sb = p_w.tile([128, MISC_W], F32, tag="misc")
            nc.sync.dma_start(out=misc_sb[:, :], in_=misc_d[:, :])
            nc.sync.dma_start(out=xs0[:, 0:KH, :], in_=xT_src[:, 0:KH, 0:512])
            nc.sync.dma_start(
                out=wqk_sb[:, 0:KH, 0:256], in_=wqk_src[:, 0:KH, 0:256]
            )
            nc.sync.dma_start(out=xs0[:, KH:KD, :], in_=xT_src[:, KH:KD, 0:512])
            nc.sync.dma_start(
                out=wqk_sb[:, KH:KD, 0:256], in_=wqk_src[:, KH:KD, 0:256]
            )

            bqk_sb = misc_sb[:, 0:NM]
            bvbc_sb = misc_sb[:, NM:NM + NH * 64]
            mask_sb = p_w.tile([128, 128], BF16, tag="mask")
            nc.vector.tensor_copy(
                mask_sb[:, :], misc_sb[:, NM + NH * 64:NM + NH * 64 + 128]
            )
            ident_sb = p_w.tile([128, 128], BF16, tag="ident")
            nc.vector.tensor_copy(
                ident_sb[:, :], misc_sb[:, NM + NH * 64 + 128:MISC_W]
            )

            wv_sb = p_w.tile([128, KD, NH * 64], BF16, tag="wv")
            wv_src = wv_d.rearrange("(c p) n -> p c n", p=128)
            nc.sync.dma_start(out=wv_sb[:, :, :], in_=wv_src[:, :, :])

            # heads 2,3's qk columns ride behind the block-0 critical loads
            nc.sync.dma_start(
                out=wqk_sb[:, :, 256:512], in_=wqk_src[:, :, 256:512]
            )

            xs1 = p_stream.tile([128, KD, 512], BF16, tag="xs", name="xs1")
            nc.sync.dma_start(out=xs1[:, :, :], in_=xT_src[:, :, 512:1024])

            # wproj is only needed by the projection fillers in the final
            # block (~90us in); load it behind everything the front needs
            wproj_sb = p_w.tile([128, NPAIR, D], BF16, tag="wproj")
            nc.sync.dma_start(out=wproj_sb[:, :, :], in_=wproj_d[:, :, :])

            qkT_sb = p_qkt.tile([128, NM, S], BF16, tag="qkt")
            v_aug = p_vaug.tile([128, NSK, NH, 65], BF16, tag="vaug")
            nc.vector.memset(v_aug[:, :, :, 64:65], 1.0)
            # attn2: pair-packed normalized attnT. partitions 0:64 head 2pp,
            # 64:128 head 2pp+1; slot pi = 2*j + pp
            attn2 = p_attn.tile([128, NPJ, 512], BF16, tag="attn")

            def load_xs(j):
                xs = p_stream.tile([128, KD, 512], BF16, tag="xs")
                nc.sync.dma_start(
                    out=xs[:, :, :], in_=xT_src[:, :, j * 512:(j + 1) * 512]
                )
                return xs

            def qk_move(j, mp, ps_qk):
                dst = qkT_sb[:, mp, j * 512:(j + 1) * 512]
                nc.vector.tensor_scalar_add(dst, ps_qk[:, :], bqk_sb[:, mp:mp + 1])

            def qk_steps(j, xs, mp):
                """Micro-steps (one matmul each) for one qk projection tile."""
                cell = {}

                def mm(k):
                    if k == 0:
                        cell["ps"] = p_misc.tile([128, 512], F32, tag="m", name="ps_qk")
                    nc.tensor.matmul(
                        cell["ps"][:, :],
                        lhsT=wqk_sb[:, k, mp * 128:(mp + 1) * 128],
                        rhs=xs[:, k, :],
                        start=(k == 0),
                        stop=(k == KD - 1),
                    )

                return [(True, lambda k=k: mm(k)) for k in range(KD)] + [
                    (False, lambda: qk_move(j, mp, cell["ps"]))
                ]

            def v_steps(j, xs, m):
                cell = {}

                def mm(k):
                    if k == 0:
                        cell["ps"] = p_misc.tile([128, NH * 64], F32, tag="m", name="ps_v")
                    nc.tensor.matmul(
                        cell["ps"][:, :],
                        lhsT=xs[:, k, (m % 4) * 128:(m % 4) * 128 + 128],
                        rhs=wv_sb[:, k, :],
                        start=(k == 0),
                        stop=(k == KD - 1),
                    )

                def mv():
                    nc.vector.tensor_add(
                        v_aug[:, m, :, 0:64],
                        cell["ps"][:, :].rearrange("p (h c) -> p h c", c=64),
                        bvbc_sb.rearrange("p (h c) -> p h c", c=64),
                    )

                return [(True, lambda k=k: mm(k)) for k in range(KD)] + [
                    (False, mv)
                ]

            def qkv_steps(j, xs):
                steps = []
                for mp in range(NM):
                    steps += qk_steps(j, xs, mp)
                for m in range(4 * j, 4 * j + 4):
                    steps += v_steps(j, xs, m)
                return steps

            def attention_block(j, fillers):
                # the final block's ACT chain is saturated; PE idle there is
                # free, so spend fillers early and keep the tail clean.
                # Early blocks have few groups but many filler steps: pump
                # hard so the next block's qk tiles (and their DVE moves)
                # finish long before the block boundary.
                reserve = [0]
                pump_n = {0: 6, 1: 3, 2: 2}.get(j, 2)

                def pump(n=1):
                    got = 0
                    while fillers and got < n:
                        if reserve[0] and sum(
                            1 for p, _ in fillers if p
                        ) <= reserve[0]:
                            return
                        is_pe, fn = fillers.pop(0)
                        fn()
                        if is_pe:
                            got += 1

                def emit_scores_h(hh, g):
                    # exact causal regions; the merged diag exp also reads
                    # stale PSUM outside them, which downstream AV never
                    # consumes (harmless garbage, skipped per subtile)
                    qTh = qkT_sb[64 * (hh % 2):64 * (hh % 2) + 64, hh // 2, :]
                    kTh = qkT_sb[64 * (hh % 2):64 * (hh % 2) + 64,
                                 NMQ + hh // 2, :]
                    ps = p_s.tile([128, 2, 512], F32, tag="s")
                    for b in range(2):
                        i = 2 * g + b
                        mb = i - 4 * j
                        no = 0 if mb <= 0 else 128 * mb
                        nc.tensor.matmul(
                            ps[:, b, no:512],
                            lhsT=kTh[:, i * 128:(i + 1) * 128],
                            rhs=qTh[:, j * 512 + no:(j + 1) * 512],
                            start=True,
                            stop=True,
                        )
                    return ps

                carry = None
                for h in range(NH):
                    ps_avq = p_avq.tile([128, 4, 65], F32, tag="avq")
                    # the 4 AV chains share one PSUM bank; a per-chain matmul
                    # start=True would mark the whole 2KB bank pending-zero
                    # and wipe its neighbours' partials. Open the bank with
                    # one zero matmul covering every chain byte, then
                    # accumulate with start=False throughout.
                    nc.tensor.matmul(
                        ps_avq[:, :, :],
                        lhsT=zero_sb[:, :],
                        rhs=ones_sb[:, :],
                        start=True,
                        stop=False,
                        skip_group_check=True,
                    )
                    pp = h // 2
                    pi = 2 * j + pp
                    if h % 2 == 0:
                        asb_cur = p_asb.tile([128, 4, 128], BF16, tag="asb")
                    asb = asb_cur
                    npair = 2 * (j + 1)
                    tail = j == SQB - 1 and h == NH - 1

                    rcp = p_rcp.tile([128, 4, 1], F32, tag="rcp")

                    def norm_subtile(t):
                        """reciprocal of the ones-column sum + normalized
                        PSUM->SBUF copy for q-subtile t of this head."""
                        nc.vector.reciprocal(
                            rcp[:, t:t + 1, :], ps_avq[:, t:t + 1, 64:65]
                        )
                        nc.vector.tensor_scalar_mul(
                            asb[:, t, 64 * (h % 2):64 * (h % 2) + 64],
                            ps_avq[:, t, 0:64],
                            rcp[:, t, :],
                        )

                    def tail_subtile(t, psT):
                        """Last head: chain t stopped one b-step ago and its
                        normalize already ran on DVE. Transpose and close
                        projection m-tile 12+t (both 512-col halves). ACT is
                        free after the final exp, so it takes the attn2 chunk
                        and one y half; DVE keeps the norms and the other y
                        half."""
                        nc.tensor.transpose(
                            psT[:, t, :], asb[:, t, :], ident_sb[:, :]
                        )
                        nc.scalar.copy(
                            attn2[:, pi, 128 * t:128 * (t + 1)], psT[:, t, :]
                        )
                        y_sb = p_y.tile([128, 2, 512], BF16, tag="y",
                                        name="y_sb")
                        chs = []
                        for n in range(2):
                            ch = p_misc.tile([128, 512], F32, tag="m",
                                             name="tp_ps")
                            chs.append(ch)
                            nc.tensor.matmul(
                                ch[:, :],
                                lhsT=attn2[:, 2 * j, t * 128:(t + 1) * 128],
                                rhs=wproj_sb[:, 0, n * 512:(n + 1) * 512],
                                start=True,
                                stop=False,
                            )
                        for n in range(2):
                            nc.tensor.matmul(
                                chs[n][:, :],
                                lhsT=attn2[:, 2 * j + 1,
                                           t * 128:(t + 1) * 128],
                                rhs=wproj_sb[:, 1, n * 512:(n + 1) * 512],
                                start=False,
                                stop=True,
                            )
                        m = 4 * j + t
                        # per-half copies split ACT/DVE; per-half DMAs let
                        # each half's descriptor-gen and transfer overlap the
                        # other half's copy
                        nc.scalar.copy(y_sb[:, 0, :], chs[0][:, :])
                        nc.sync.dma_start(
                            out=y_d[m * 128:(m + 1) * 128, 0:512],
                            in_=y_sb[:, 0, :],
                        )
                        nc.vector.tensor_copy(y_sb[:, 1, :], chs[1][:, :])
                        nc.sync.dma_start(
                            out=y_d[m * 128:(m + 1) * 128, 512:1024],
                            in_=y_sb[:, 1, :],
                        )

                    sc_next = carry if carry is not None else emit_scores_h(h, 0)
                    carry = None
                    if tail:
                        psT = p_pst.tile([128, 4, 128], BF16, tag="pst")
                    for g in range(npair):
                        ps_sc = sc_next
                        # 1-deep software pipeline: next group's scores are
                        # emitted before this group's AV so PE runs them
                        # while ACT computes this group's exp. The pipeline
                        # carries across heads (same block) so ACT never
                        # bubbles at a head boundary.
                        if g + 1 < npair:
                            sc_next = emit_scores_h(h, g + 1)
                        elif h + 1 < NH:
                            carry = emit_scores_h(h + 1, 0)
                        # ACT runs ~350ns/group longer than this loop's PE
                        # work; pump filler steps so PE never idles on exp
                        pump(pump_n)
                        exp_t = p_exp.tile([128, 2, 512], BF16, tag="exp")
                        if g == 2 * j:
                            # diag pair mb=0,1: single exp over both tiles
                            nc.scalar.activation(
                                exp_t[:, :, :], ps_sc[:, :, :], AFT.Exp, scale=SCALE
                            )
                            nc.vector.tensor_mul(
                                exp_t[:, 0, 0:128], exp_t[:, 0, 0:128], mask_sb[:, :]
                            )
                            nc.vector.tensor_mul(
                                exp_t[:, 1, 128:256], exp_t[:, 1, 128:256],
                                mask_sb[:, :],
                            )
                        elif g == 2 * j + 1:
                            # mb=2,3: exp the computed 256:512 of both tiles
                            nc.scalar.activation(
                                exp_t[:, :, 256:512],
                                ps_sc[:, :, 256:512],
                                AFT.Exp,
                                scale=SCALE,
                            )
                            nc.vector.tensor_mul(
                                exp_t[:, 0, 256:384], exp_t[:, 0, 256:384],
                                mask_sb[:, :],
                            )
                            nc.vector.tensor_mul(
                                exp_t[:, 1, 384:512], exp_t[:, 1, 384:512],
                                mask_sb[:, :],
                            )
                        else:
                            nc.scalar.activation(
                                exp_t[:, :, :], ps_sc[:, :, :], AFT.Exp, scale=SCALE
                            )
                        for b in range(2):
                            i = 2 * g + b
                            mb = i - 4 * j
                            for t in range(max(0, mb), 4):
                                nc.tensor.matmul(
                                    ps_avq[:, t, :],
                                    lhsT=exp_t[:, b, 128 * t:128 * (t + 1)],
                                    rhs=v_aug[:, i, h, :],
                                    start=False,
                                    stop=(i == 4 * j + t),
                                    skip_group_check=True,
                                )
                            if tail and mb >= 0:
                                # chain mb just stopped: normalize on DVE now
                                norm_subtile(mb)
                            if tail and mb >= 1:
                                # chain mb-1 normalized one b-step ago ->
                                # transpose + close its projection m-tile
                                tail_subtile(mb - 1, psT)
                    if tail:
                        # drain remaining fillers (their y DMAs must precede
                        # the final m-tile's), then close the last subtile
                        reserve[0] = 0
                        while fillers:
                            fillers.pop(0)[1]()
                        tail_subtile(3, psT)
                    elif h % 2 == 0:
                        for t in range(4):
                            norm_subtile(t)
                    else:
                        psT = p_pst.tile([128, 4, 128], BF16, tag="pst")
                        for t in range(4):
                            norm_subtile(t)
                        # fillers between the DVE normalizes and the PE
                        # transposes hide the normalize latency
                        pump(2)
                        for t in range(4):
                            nc.tensor.transpose(
                                psT[:, t, :], asb[:, t, :], ident_sb[:, :]
                            )
                        nc.vector.tensor_copy(attn2[:, pi, :], psT[:, :, :])
                    # drain PE filler work into the ACT-paced stretch,
                    # counting only PE (matmul) steps toward the quota
                    if h >= 1 and not tail:
                        npe = sum(1 for is_pe, _ in fillers if is_pe)
                        take = max(1, (npe - reserve[0]) // (6 * (NH - h)))
                        while fillers and take > 0:
                            if reserve[0] and sum(
                                1 for p, _ in fillers if p
                            ) <= reserve[0]:
                                break
                            is_pe, fn = fillers.pop(0)
                            fn()
                            if is_pe:
                                take -= 1
                while fillers:
                    fillers.pop(0)[1]()

            def proj_steps_m(j, m):
                """Micro-steps for one 128-row tile of the out-projection.
                Each 512-col chain (both pair accumulations) is one atomic
                step so a pump boundary never leaves a PSUM chain open while
                other code allocates from the same pool."""
                o = (m % 4) * 128
                cell = {}

                def mmv(n):
                    if n == 0:
                        cell["y"] = p_y.tile([128, 2, 512], BF16, tag="y",
                                             name="y_sb")
                    ps = p_misc.tile([128, 512], F32, tag="m", name="ps_y")
                    for pp in range(NPAIR):
                        nc.tensor.matmul(
                            ps[:, :],
                            lhsT=attn2[:, 2 * j + pp, o:o + 128],
                            rhs=wproj_sb[:, pp, n * 512:(n + 1) * 512],
                            start=(pp == 0),
                            stop=(pp == NPAIR - 1),
                        )
                    nc.vector.tensor_copy(cell["y"][:, n, :], ps[:, :])

                def out():
                    nc.sync.dma_start(
                        out=y_d[m * 128:(m + 1) * 128, :],
                        in_=cell["y"][:, :, :],
                    )

                steps = []
                for n in range(2):
                    steps.append((True, lambda n=n: mmv(n)))
                steps.append((False, out))
                return steps

            def proj_steps(j):
                steps = []
                for m in range(j * 4, j * 4 + 4):
                    steps += proj_steps_m(j, m)
                return steps

            # j=0 prologue. The startup is DMA-serial-bound: run all four qk
            # tiles chunk-half-major (4 concurrent PSUM chains, borrowing the
            # idle score pool) so PE tracks the half-chunk DMA cadence; then
            # v chunk-major the same way.
            junk(75)
            ps_qk0 = p_misc.tile([128, 512], F32, tag="m")
            ps_qk1 = p_misc.tile([128, 512], F32, tag="m")
            ps_qk23 = p_s.tile([128, 2, 512], F32, tag="s")
            chains = (ps_qk0[:, :], ps_qk1[:, :], ps_qk23[:, 0, :],
                      ps_qk23[:, 1, :])
            for kh in range(2):
                for k in range(kh * KH, (kh + 1) * KH):
                    for mp in range(NM):
                        nc.tensor.matmul(
                            chains[mp],
                            lhsT=wqk_sb[:, k, mp * 128:(mp + 1) * 128],
                            rhs=xs0[:, k, :],
                            start=(k == 0),
                            stop=(k == KD - 1),
                        )
            for mp in range(NM):
                qk_move(0, mp, chains[mp])
            ps_v01 = p_s.tile([128, 2, 512], F32, tag="s")
            vchains = (ps_v01[:, 0, 0:256], ps_v01[:, 1, 0:256], None, None)
            vcells = [None, None, None, None]
            for k in range(KD):
                for m in range(4):
                    if m < 2:
                        ps = vchains[m]
                    else:
                        if k == 0 and vcells[m] is None:
                            vcells[m] = p_misc.tile(
                                [128, NH * 64], F32, tag="m", name="ps_v"
                            )
                        ps = vcells[m][:, :]
                    nc.tensor.matmul(
                        ps,
                        lhsT=xs0[:, k, m * 128:m * 128 + 128],
                        rhs=wv_sb[:, k, :],
                        start=(k == 0),
                        stop=(k == KD - 1),
                    )
            for m in range(4):
                src = vchains[m] if m < 2 else vcells[m][:, :]
                nc.vector.tensor_add(
                    v_aug[:, m, :, 0:64],
                    src.rearrange("p (h c) -> p h c", c=64),
                    bvbc_sb.rearrange("p (h c) -> p h c", c=64),
                )
            xs_next = xs1 if SQB > 1 else None
            for j in range(SQB):
                fillers = []
                if j + 1 < SQB:
                    fillers += qkv_steps(j + 1, xs_next)
                    xs_after = load_xs(j + 2) if j + 2 < SQB else None
                else:
                    xs_after = None
                if j == SQB - 1:
                    for jp in range(SQB - 1):
                        fillers += proj_steps(jp)
                attention_block(j, fillers)
                xs_next = xs_after

    return nc


def make_mask4():
    p = np.arange(128)[:, None]
    f = np.arange(128)[None, :]
    return (f >= p).astype(np.float32).copy()  # [128, 128] lower-tri in T layout


def to_bf16(x):
    import ml_dtypes

    return np.asarray(x, dtype=np.float32).astype(ml_dtypes.bfloat16)


def make_in_maps(x, W_qkv, b_qkv, W_proj):
    """Per-core input dicts for the full-size problem (bf16 staged)."""
    mask4 = make_mask4()
    ident = np.eye(128, dtype=np.float32)
    in_maps = []
    for c in range(NCORES):
        b, q = c // 4, c % 4
        cq = slice(256 * q, 256 * q + 256)
        wqk = np.concatenate([W_qkv[:, cq], W_qkv[:, 1024:2048][:, cq]], axis=1)
        wv = W_qkv[:, 2048:3072][:, cq]
        bqk = np.concatenate([b_qkv[cq], b_qkv[1024:2048][cq]]).reshape(4, 128)
        bvbc = np.broadcast_to(b_qkv[2048:3072][cq], (128, 256))
        # packed misc input: [128, 4 bqk-cols | 256 bvbc | 128 mask | 128 id]
        misc = np.concatenate([bqk.T, bvbc, mask4, ident], axis=1).astype(
            np.float32
        )
        # pair-packed wproj: [p, pp, n] = W_proj[256*q + pp*128 + p, n]
        wproj = np.ascontiguousarray(
            W_proj[cq, :].reshape(2, 128, 1024).transpose(1, 0, 2)
        )
        in_maps.append(
            {
                "xT": np.ascontiguousarray(to_bf16(x[b].T)),
                "wqk": np.ascontiguousarray(to_bf16(wqk)),
                "wv": np.ascontiguousarray(to_bf16(wv)),
                "misc": np.ascontiguousarray(misc),
                "wproj": to_bf16(wproj),
            }
        )
    return in_maps


_NC_CACHE = {}


def _get_nc():
    if "nc" not in _NC_CACHE:
        _NC_CACHE["nc"] = build_nc()
    return _NC_CACHE["nc"]


def run_on_hw(x, W_qkv, b_qkv, W_proj, b_proj, trace=False, **trace_kw):
    from concourse.bass_utils import run_bass_kernel_spmd

    in_maps = make_in_maps(x, W_qkv, b_qkv, W_proj)
    res = run_bass_kernel_spmd(
        _get_nc(), in_maps, core_ids=list(range(NCORES)), trace=trace, **trace_kw
    )
    out = np.empty((B, S, D), dtype=np.float32)
    for b in range(B):
        acc = res.results[4 * b]["y"].astype(np.float32)
        for q in range(1, 4):
            acc = acc + res.results[4 * b + q]["y"].astype(np.float32)
        out[b] = acc + b_proj[None, :]
    return out, res


def kernel(x, W_qkv, b_qkv, W_proj, b_proj):
    x = np.asarray(x, dtype=np.float32)
    W_qkv = np.asarray(W_qkv, dtype=np.float32)
    b_qkv = np.asarray(b_qkv, dtype=np.float32)
    W_proj = np.asarray(W_proj, dtype=np.float32)
    b_proj = np.asarray(b_proj, dtype=np.float32)
    out, _ = run_on_hw(x, W_qkv, b_qkv, W_proj, b_proj, trace=False)
    return out
